# revision 2
# baseline (speedup 1.0000x reference)
"""Trainium2 Bass kernel for nn_CNVRegressor (CNN tokenizer + 5-layer Performer + head).

Sharding: data-parallel over batch B=16 across 8 cores (2 samples/core).
Layout: feature-major activations [D on partitions, tokens on free].
Per-sample sequence padded 1001 -> 1024; two samples side by side -> [512, 2048].
Per-layer cross-core AllReduce(max) reproduces the reference's GLOBAL key-stab.

Self-contained: hardcodes all shapes; host does only input prep / sharding
(cleanup, halo pad, embedding row gather, PE table, bias/mask packing).
"""
import math
from contextlib import ExitStack

import ml_dtypes
import numpy as np

import concourse.bass as bass
import concourse.bacc as bacc
import concourse.tile as tile
from concourse import mybir
from concourse.bass_isa import ReduceOp
from concourse.bass_utils import run_bass_kernel_spmd
from concourse.masks import make_identity

F32 = mybir.dt.float32
F32R = mybir.dt.float32r
BF16 = mybir.dt.bfloat16
AF = mybir.ActivationFunctionType
OP = mybir.AluOpType
AX = mybir.AxisListType

P = 128
D = 512
DH = 64
DEPTH = 5
L = 1000
M = 266
NPAD = 1024
N2 = 2 * NPAD
NT = NPAD // P            # 8 token tiles per sample
DC = D // P               # 4 feature chunks
BLK = 512
DN = DH ** -0.25
DNS = DN * math.sqrt(0.5)
LN_EPS = 1e-5
GN_EPS = 1e-5
LOG_EPS = float(np.log(1e-4))
MCH = ((0, 128), (128, 128), (256, 10))   # m-chunks of 266
N_CORES = 8


def build_cols():
    cols, idx = {}, 0

    def a(name, n):
        nonlocal idx
        cols[name] = idx
        idx += n

    for n in ('gn_w', 'gn_b', 'lnt_w', 'lnt_b'):
        a(n, 4)
    a('b1', 1); a('b2', 1); a('b3', 4); a('gmask', 8)
    a('real', 16); a('stabb', 16); a('vmask', 16)
    for n in ('hln_w', 'hln_b', 'hb1'):
        a(n, 4)
    a('hb2', 1)
    a('cln', 1)
    a('clog', 1)
    for l in range(DEPTH):
        for n in ('ln1w', 'ln1b', 'ln2w', 'ln2b', 'bq', 'bk', 'bv', 'bo', 'fb2'):
            a(f'{n}{l}', 4)
        a(f'fb1{l}', 16)
    return cols, idx


COLS, NCOL = build_cols()


def pack_chw(inp, real01, vmask01):
    chw = np.zeros((P, NCOL), np.float32)

    def put(name, vec):
        vec = np.asarray(vec, np.float32).reshape(-1)
        c0 = COLS[name]
        for c in range((len(vec) + P - 1) // P):
            seg = vec[c * P:(c + 1) * P]
            chw[:len(seg), c0 + c] = seg

    put('gn_w', inp['gn_w']); put('gn_b', inp['gn_b'])
    put('lnt_w', inp['lnt_w']); put('lnt_b', inp['lnt_b'])
    put('b1', inp['conv1_b']); put('b2', inp['conv2_b']); put('b3', inp['conv3_b'])
    gm = np.zeros((P, 8), np.float32)
    for p in range(P):
        gm[p, p // 16] = 1.0
    chw[:, COLS['gmask']:COLS['gmask'] + 8] = gm
    chw[:, COLS['real']:COLS['real'] + 16] = real01
    chw[:, COLS['stabb']:COLS['stabb'] + 16] = (real01 - 1.0) * 1e30
    chw[:, COLS['vmask']:COLS['vmask'] + 16] = vmask01
    put('hln_w', inp['h_ln_w']); put('hln_b', inp['h_ln_b'])
    put('hb1', inp['h_b1']); put('hb2', inp['h_b2'])
    chw[:, COLS['cln']] = LN_EPS
    chw[:, COLS['clog']] = LOG_EPS
    for l in range(DEPTH):
        put(f'ln1w{l}', inp['ln1_w'][l]); put(f'ln1b{l}', inp['ln1_b'][l])
        put(f'ln2w{l}', inp['ln2_w'][l]); put(f'ln2b{l}', inp['ln2_b'][l])
        put(f'bq{l}', inp['bq'][l]); put(f'bk{l}', inp['bk'][l])
        put(f'bv{l}', inp['bv'][l]); put(f'bo{l}', inp['bo'][l])
        put(f'fb1{l}', inp['ff_b1'][l]); put(f'fb2{l}', inp['ff_b2'][l])
    return chw


def blocks(width, bs=BLK, off0=0):
    out, off = [], 0
    while off < width:
        out.append((off0 + off, min(bs, width - off)))
        off += bs
    return out


# ---------------------------------------------------------------- device build
def build(nc):
    r = lambda ap: ap.bitcast(F32R)

    dp = lambda n, sh, dt=F32: nc.declare_dram_parameter(n, sh, dt, isOutput=False)
    xh_d = dp('xh', (2, 36, NPAD))
    add_d = dp('addt', (2, D, NPAD))
    w1t_d = dp('w1t', (36, 64))
    w2t_d = dp('w2t', (64, 9, 128))
    w3t_d = dp('w3t', (128, D))
    chw_d = dp('chw', (P, NCOL))
    gmt_d = dp('gmt', (8, P))
    proj_d = dp('projt', (DEPTH, DH, M))
    wq_d = dp('wqb', (DEPTH, D, D), BF16)
    wk_d = dp('wkb', (DEPTH, D, D), BF16)
    wv_d = dp('wvb', (DEPTH, D, D), BF16)
    wob_d = dp('wob', (DEPTH, D, D), BF16)
    f1_d = dp('f1b', (DEPTH, D, 4 * D), BF16)
    f2_d = dp('f2b', (DEPTH, 4 * D, D), BF16)
    hw1_d = dp('hw1b', (D, D), BF16)
    hw2_d = dp('hw2', (D, 1))
    out_d = nc.declare_dram_parameter('o', (1, 2), F32, isOutput=True)

    with tile.TileContext(nc) as tc, ExitStack() as ctx:
        const = ctx.enter_context(tc.tile_pool(name='const', bufs=1))
        sp = ctx.enter_context(tc.tile_pool(name='sp', bufs=1))
        actp = ctx.enter_context(tc.tile_pool(name='actp', bufs=1))
        wpp = ctx.enter_context(tc.tile_pool(name='wpp', bufs=2))
        fp = ctx.enter_context(tc.tile_pool(name='fp', bufs=2))
        kvp = ctx.enter_context(tc.tile_pool(name='kvp', bufs=3))
        hfp = ctx.enter_context(tc.tile_pool(name='hfp', bufs=1))
        dram = ctx.enter_context(tc.tile_pool(name='dram', bufs=2, space='DRAM'))
        pmm = ctx.enter_context(tc.tile_pool(name='pmm', bufs=3, space='PSUM'))
        ps1 = ctx.enter_context(tc.tile_pool(name='ps1', bufs=3, space='PSUM'))

        # ---- constants
        chw = const.tile([P, NCOL], F32, name='chw')
        nc.sync.dma_start(chw[:], chw_d[:])
        cw = lambda name, off=0: chw[:, COLS[name] + off:COLS[name] + off + 1]
        cwp = lambda name, parts: chw[0:parts, COLS[name]:COLS[name] + 1]
        ident = const.tile([P, P], F32, name='ident')
        make_identity(nc, ident)
        identb = const.tile([P, P], BF16, name='identb')
        nc.vector.tensor_copy(identb[:], ident[:])
        ones = const.tile([P, 1], F32, name='ones')
        nc.vector.memset(ones[:], 1.0)
        onesb = const.tile([P, 1], BF16, name='onesb')
        nc.vector.memset(onesb[:], 1.0)
        gmt = const.tile([8, P], F32, name='gmt')
        nc.sync.dma_start(gmt[:], gmt_d[:])
        w1t = const.tile([36, 64], F32, name='w1t')
        nc.sync.dma_start(w1t[:], w1t_d[:])
        w2t = const.tile([64, 9, 128], F32, name='w2t')
        nc.sync.dma_start(w2t[:], w2t_d[:])
        w3t = const.tile([128, D], F32, name='w3t')
        nc.sync.dma_start(w3t[:], w3t_d[:])

        S = [sp.tile([P, N2], F32, tag=f'S{c}', name=f'S{c}') for c in range(DC)]

        # -------------------------------------------------------- shared LN
        def ln_fm(X, wc, bc, col0, width, ytag):
            """Per-token LN over the 512 partition dim (feature-major).

            X tiles are f32; returns bf16 normed tiles."""
            Y = [actp.tile([P, N2], BF16, tag=f'{ytag}{c}', name=f'{ytag}{c}')
                 for c in range(DC)]
            for c in range(DC):
                nc.vector.tensor_mul(Y[c][:, col0:col0 + width],
                                     X[c][:, col0:col0 + width],
                                     X[c][:, col0:col0 + width])
            srow = fp.tile([1, N2], F32, tag='srow', bufs=1, name='srow')
            qrow = fp.tile([1, N2], F32, tag='qrow', bufs=1, name='qrow')
            trow = fp.tile([1, N2], F32, tag='trow', bufs=1, name='trow')
            mrow, vrow = srow, qrow
            for o, bw in blocks(width, BLK, col0):
                ps = ps1.tile([1, BLK], F32, tag='ps1', name='ps')
                pq = ps1.tile([1, BLK], F32, tag='ps1', name='pq')
                for c in range(DC):
                    nc.tensor.matmul(ps[:, :bw], ones[:], X[c][:, o:o + bw],
                                     start=(c == 0), stop=(c == DC - 1))
                    nc.tensor.matmul(pq[:, :bw], onesb[:], Y[c][:, o:o + bw],
                                     start=(c == 0), stop=(c == DC - 1))
                nc.any.tensor_copy(srow[:, o:o + bw], ps[:, :bw])
                nc.any.tensor_copy(qrow[:, o:o + bw], pq[:, :bw])
            sl = slice(col0, col0 + width)
            nc.vector.tensor_scalar_mul(mrow[:, sl], srow[:, sl], 1.0 / D)
            nc.vector.tensor_mul(trow[:, sl], mrow[:, sl], mrow[:, sl])
            nc.vector.tensor_scalar(vrow[:, sl], qrow[:, sl], 1.0 / D, None,
                                    OP.mult)
            nc.vector.tensor_sub(vrow[:, sl], vrow[:, sl], trow[:, sl])
            nc.scalar.activation(vrow[:, sl], vrow[:, sl], AF.Ln,
                                 bias=cwp('cln', 1))
            nc.scalar.activation(vrow[:, sl], vrow[:, sl], AF.Exp, scale=-0.5)
            MU = actp.tile([P, N2], F32, tag='MU', name='MU')
            RS = actp.tile([P, N2], F32, tag='RS', name='RS')
            nc.gpsimd.partition_broadcast(MU[:, sl], mrow[:, sl], P)
            nc.gpsimd.partition_broadcast(RS[:, sl], vrow[:, sl], P)
            for c in range(DC):
                nc.vector.tensor_sub(Y[c][:, sl], X[c][:, sl], MU[:, sl])
                nc.vector.tensor_mul(Y[c][:, sl], Y[c][:, sl], RS[:, sl])
                nc.scalar.activation(Y[c][:, sl], Y[c][:, sl], AF.Identity,
                                     scale=cw(wc, c), bias=cw(bc, c))
            return Y

        # head-pair qkv matmul helper: evicts psum into per-head [64, N2] tiles
        def pair_mm(wd, l, hp, bn, dsts):
            wt = wpp.tile([P, DC, P], BF16, tag='wpq', name='wt')
            nc.sync.dma_start(
                wt[:], wd[l, :, hp * P:(hp + 1) * P]
                .rearrange('(kc p) m -> p kc m', p=P))
            for b, bw in blocks(N2):
                pm = pmm.tile([P, BLK], F32, tag='pmm', name='pm')
                for kc in range(DC):
                    nc.tensor.matmul(pm[:], wt[:, kc, :], y[kc][:, b:b + bw],
                                     start=(kc == 0), stop=(kc == DC - 1))
                for hh in range(2):
                    bias = chw[hh * DH:(hh + 1) * DH,
                               COLS[f'{bn}{l}'] + hp:COLS[f'{bn}{l}'] + hp + 1]
                    nc.scalar.activation(dsts[hh][:, b:b + bw],
                                         pm[hh * DH:(hh + 1) * DH, :],
                                         AF.Identity, bias=bias)

        # -------------------------------------------------------- tokenizer
        for s in range(2):
            for c in range(DC):
                nc.sync.dma_start(S[c][:, s * NPAD:(s + 1) * NPAD],
                                  add_d[s, c * P:(c + 1) * P, :])
        for s in range(2):
            t1 = fp.tile([36, NPAD], F32, tag='tokA', bufs=1, name='t1')
            nc.sync.dma_start(t1[:], xh_d[s])
            y1h = fp.tile([64, L + 8], F32, tag='tokB', bufs=1, name='y1h')
            nc.vector.memset(y1h[:], 0.0)
            for o, bw in blocks(L):
                p1 = pmm.tile([64, BLK], F32, tag='pmm', name='p1')
                nc.tensor.matmul(p1[:, :bw], w1t[:], t1[:, o:o + bw],
                                 start=True, stop=True)
                nc.scalar.activation(y1h[:, 4 + o:4 + o + bw], p1[:, :bw], AF.Gelu,
                                     bias=chw[0:64, COLS['b1']:COLS['b1'] + 1])
            y2 = fp.tile([P, NPAD], F32, tag='tokA', bufs=1, name='y2')
            for o, bw in blocks(L):
                p2 = pmm.tile([P, BLK], F32, tag='pmm', name='p2')
                for t in range(9):
                    nc.tensor.matmul(p2[:, :bw], w2t[:, t, :],
                                     y1h[:, t + o:t + o + bw],
                                     start=(t == 0), stop=(t == 8))
                nc.scalar.activation(y2[:, o:o + bw], p2[:, :bw], AF.Gelu,
                                     bias=cw('b2'))
            x3 = [actp.tile([P, NPAD], F32, tag=f'A{c}', name=f'x3{c}')
                  for c in range(DC)]
            for c in range(DC):
                for o, bw in blocks(L):
                    p3 = pmm.tile([P, BLK], F32, tag='pmm', name='p3')
                    nc.tensor.matmul(p3[:, :bw], w3t[:, c * P:(c + 1) * P],
                                     y2[:, o:o + bw], start=True, stop=True)
                    nc.scalar.activation(x3[c][:, o:o + bw], p3[:, :bw],
                                         AF.Identity, bias=cw('b3', c))
            # GroupNorm(32, 512) over [16ch x 1000]
            stats = fp.tile([P, 8], F32, tag='gstats', name='stats')
            sqt = fp.tile([P, NPAD], F32, tag='tokB', bufs=1, name='sqt')
            for c in range(DC):
                nc.vector.tensor_reduce(stats[:, c:c + 1], x3[c][:, 0:L], AX.X, OP.add)
                nc.vector.tensor_mul(sqt[:, 0:L], x3[c][:, 0:L], x3[c][:, 0:L])
                nc.vector.tensor_reduce(stats[:, 4 + c:5 + c], sqt[:, 0:L], AX.X, OP.add)
            pg = ps1.tile([8, 8], F32, tag='ps1', name='pg')
            nc.tensor.matmul(pg[:], chw[:, COLS['gmask']:COLS['gmask'] + 8],
                             stats[:], start=True, stop=True)
            gs = fp.tile([8, 8], F32, tag='gs', name='gs')
            nc.vector.tensor_scalar_mul(gs[:], pg[:], 1.0 / (16 * L))
            gm2 = fp.tile([8, 4], F32, tag='gm2', name='gm2')
            nc.vector.tensor_mul(gm2[:], gs[:, 0:4], gs[:, 0:4])
            nc.vector.tensor_sub(gs[:, 4:8], gs[:, 4:8], gm2[:])
            nc.scalar.activation(gs[:, 4:8], gs[:, 4:8], AF.Ln, bias=cwp('cln', 8))
            nc.scalar.activation(gs[:, 4:8], gs[:, 4:8], AF.Exp, scale=-0.5)
            pb = ps1.tile([P, 8], F32, tag='ps1', name='pb')
            nc.tensor.matmul(pb[:], gmt[:], gs[:], start=True, stop=True)
            cstat = fp.tile([P, 8], F32, tag='cstat', name='cstat')
            nc.vector.tensor_copy(cstat[:], pb[:])
            for c in range(DC):
                nc.vector.tensor_scalar(x3[c][:, 0:L], x3[c][:, 0:L],
                                        cstat[:, c:c + 1], cstat[:, 4 + c:5 + c],
                                        OP.subtract, OP.mult)
                nc.scalar.activation(x3[c][:, 0:L], x3[c][:, 0:L], AF.Identity,
                                     scale=cw('gn_w', c), bias=cw('gn_b', c))
            tok = ln_fm(x3, 'lnt_w', 'lnt_b', 0, L, 'y')
            b0 = s * NPAD
            for c in range(DC):
                nc.vector.tensor_add(S[c][:, b0 + 1:b0 + 1 + L],
                                     S[c][:, b0 + 1:b0 + 1 + L], tok[c][:, 0:L])

        # -------------------------------------------------------- layers
        for l in range(DEPTH):
            projT = fp.tile([DH, M], F32, tag='projT', name='projT')
            nc.sync.dma_start(projT[:], proj_d[l])
            projTb = fp.tile([DH, M], BF16, tag='projTb', name='projTb')
            nc.vector.tensor_copy(projTb[:], projT[:])

            y = ln_fm(S, f'ln1w{l}', f'ln1b{l}', 0, N2, 'y')

            # ---- pass A: k + global key-stab scan
            smax = fp.tile([P, 16], F32, tag='smax', name='smax')
            for hp in range(4):
                kh = [fp.tile([DH, N2], BF16, tag=f'kh{hh}', bufs=1, name=f'kh{hh}')
                      for hh in range(2)]
                pair_mm(wk_d, l, hp, 'bk', kh)
                for hh in range(2):
                    for s in range(2):
                        rm8 = fp.tile([P, 8], F32, tag='rm8', name='rm8')
                        for t in range(NT):
                            csl = slice(s * NPAD + t * P, s * NPAD + (t + 1) * P)
                            pdk = ps1.tile([P, 272], F32, tag='ps1', name='pdk')
                            nc.tensor.matmul(pdk[:, 0:M], kh[hh][:, csl],
                                             projTb[:], start=True, stop=True)
                            nc.vector.tensor_reduce(rm8[:, t:t + 1], pdk[:, 0:M],
                                                    AX.X, OP.max)
                        c16 = COLS['real'] + s * 8
                        nc.vector.tensor_mul(rm8[:], rm8[:], chw[:, c16:c16 + 8])
                        c16 = COLS['stabb'] + s * 8
                        nc.vector.tensor_add(rm8[:], rm8[:], chw[:, c16:c16 + 8])
                        i16 = hp * 4 + hh * 2 + s
                        nc.vector.tensor_reduce(smax[:, i16:i16 + 1], rm8[:],
                                                AX.X, OP.max)
            sfin = fp.tile([P, 1], F32, tag='sfin', name='sfin')
            nc.vector.tensor_reduce(sfin[:], smax[:], AX.X, OP.max)
            nc.gpsimd.partition_all_reduce(sfin[:], sfin[:], P, ReduceOp.max)
            bin_ = dram.tile([P, 1], F32, name='bin')
            bout = dram.tile([P, 1], F32, name='bout')
            nc.sync.dma_start(bin_[:], sfin[:])
            nc.gpsimd.collective_compute(
                'AllReduce', OP.max,
                replica_groups=[list(range(N_CORES))],
                ins=[bin_.opt()], outs=[bout.opt()])
            stabg = fp.tile([P, 1], F32, tag='stabg', name='stabg')
            nc.sync.dma_start(stabg[:], bout[:])
            ceps65 = fp.tile([65, 1], F32, tag='ceps65', name='ceps65')
            nc.scalar.activation(ceps65[:], stabg[0:65, :], AF.Exp,
                                 bias=cwp('clog', 65))

            # ---- pass B: q/k/v per head-pair + FAVOR+
            A = [actp.tile([P, N2], BF16, tag=f'A{c}', name=f'Aa{c}')
                 for c in range(DC)]
            for hp in range(4):
                qh = [fp.tile([DH, N2], BF16, tag=f'qh{hh}', bufs=1, name=f'qh{hh}')
                      for hh in range(2)]
                kh = [fp.tile([DH, N2], BF16, tag=f'kh{hh}', bufs=1, name=f'khB{hh}')
                      for hh in range(2)]
                vh = [fp.tile([DH, N2], BF16, tag=f'vh{hh}', bufs=1, name=f'vh{hh}')
                      for hh in range(2)]
                pair_mm(wq_d, l, hp, 'bq', qh)
                pair_mm(wk_d, l, hp, 'bk', kh)
                pair_mm(wv_d, l, hp, 'bv', vh)
                for hh in range(2):
                    for s in range(2):
                        base = s * NPAD
                        # --- k side: diag_k then kp = exp(ddk - diagk)
                        biask = fp.tile([P, 8], F32, tag='biask', name='biask')
                        for t in range(NT):
                            csl = slice(base + t * P, base + (t + 1) * P)
                            pkt = ps1.tile([P, 64], BF16, tag='ps1', name='pkt')
                            nc.tensor.transpose(pkt[:], kh[hh][:, csl],
                                                identb[0:64, 0:64])
                            sqk = fp.tile([P, 64], F32, tag='sqk', name='sqk')
                            nc.scalar.activation(sqk[:], pkt[:], AF.Square,
                                                 scale=DNS)
                            nc.vector.tensor_reduce(biask[:, t:t + 1], sqk[:],
                                                    AX.X, OP.add)
                        nc.vector.tensor_scalar_mul(biask[:], biask[:], -1.0)
                        # --- ctx'^T [65, 266] and vsum [65, 1], accumulated per tile
                        pctx = ps1.tile([65, M], F32, tag='psx', bufs=1, name='pctx')
                        pvs = ps1.tile([65, 1], F32, tag='psv', bufs=1, name='pvs')
                        for t in range(NT):
                            csl = slice(base + t * P, base + (t + 1) * P)
                            pdk = ps1.tile([P, 272], F32, tag='ps1', name='pdkB')
                            nc.tensor.matmul(pdk[:, 0:M], kh[hh][:, csl],
                                             projTb[:], start=True, stop=True)
                            kp = kvp.tile([P, M], BF16, tag='kp', name='kp')
                            nc.scalar.activation(kp[:], pdk[:, 0:M], AF.Exp,
                                                 bias=biask[:, t:t + 1])
                            pvt = ps1.tile([P, 64], BF16, tag='ps1', name='pvt')
                            nc.tensor.transpose(pvt[:], vh[hh][:, csl],
                                                identb[0:64, 0:64])
                            v1 = kvp.tile([P, 65], BF16, tag='v1', name='v1')
                            nc.vector.tensor_scalar_mul(v1[:, 0:64], pvt[:],
                                                        cw('vmask', s * 8 + t))
                            nc.vector.tensor_copy(v1[:, 64:65],
                                                  cw('real', s * 8 + t))
                            nc.tensor.matmul(pctx[:], v1[:], kp[:],
                                             start=(t == 0), stop=(t == NT - 1))
                            nc.tensor.matmul(pvs[:], v1[:], onesb[:],
                                             start=(t == 0), stop=(t == NT - 1))
                        vsc = fp.tile([65, 1], F32, tag='vsc', name='vsc')
                        nc.vector.tensor_mul(vsc[:], pvs[:], ceps65[:])
                        ctxT = fp.tile([65, M], BF16, tag='ctxT', name='ctxT')
                        nc.vector.tensor_scalar(ctxT[:], pctx[:], vsc[:], None,
                                                OP.add)
                        # --- ctx -> [266, 65] chunks; colsum [1, 65]
                        ctx_sb = []
                        for ci, (m0, mw) in enumerate(MCH):
                            ptc = ps1.tile([P, 65], BF16, tag='ps1', name='ptc')
                            nc.tensor.transpose(ptc[0:mw, :], ctxT[:, m0:m0 + mw],
                                                identb[0:65, 0:65])
                            csb = fp.tile([P, 65], BF16, tag=f'ctx{ci}', name=f'c{ci}')
                            nc.any.tensor_copy(csb[0:mw, :], ptc[0:mw, :])
                            ctx_sb.append(csb)
                        pcs = ps1.tile([1, 65], F32, tag='ps1', name='pcs')
                        for ci, (m0, mw) in enumerate(MCH):
                            nc.tensor.matmul(pcs[:], onesb[0:mw, :],
                                             ctx_sb[ci][0:mw, :],
                                             start=(ci == 0), stop=(ci == 2))
                        csr = fp.tile([1, 65], BF16, tag='csr', name='csr')
                        nc.any.tensor_copy(csr[:], pcs[:])
                        # --- q side: qp = exp(ddq), feature-major
                        qp = [fp.tile([P, NPAD], BF16, tag='qp0', bufs=1, name='qp0'),
                              fp.tile([P, NPAD], BF16, tag='qp1', bufs=1, name='qp1'),
                              fp.tile([10, NPAD], BF16, tag='qp2', bufs=1, name='qp2')]
                        for ci, (m0, mw) in enumerate(MCH):
                            for b, bw in blocks(NPAD):
                                pdq = pmm.tile([P, BLK], F32, tag='pmm', name='pdq')
                                nc.tensor.matmul(
                                    pdq[0:mw, :], projTb[:, m0:m0 + mw],
                                    qh[hh][:, base + b:base + b + bw],
                                    start=True, stop=True)
                                nc.scalar.activation(qp[ci][0:mw, b:b + bw],
                                                     pdq[0:mw, :], AF.Exp)
                        # --- colmax = e^{stab_q} per token
                        ar = fp.tile([P, NPAD], BF16, tag='ar', bufs=1, name='ar')
                        epsE = fp.tile([1, NPAD], BF16, tag='epsE', bufs=1,
                                       name='epsE')
                        cm = fp.tile([1, NPAD], BF16, tag='cmx', bufs=1, name='cmx')
                        nc.gpsimd.partition_all_reduce(ar[:], qp[0][:], P,
                                                       ReduceOp.max)
                        nc.vector.tensor_copy(cm[:], ar[0:1, :])
                        nc.gpsimd.partition_all_reduce(ar[:], qp[1][:], P,
                                                       ReduceOp.max)
                        nc.vector.tensor_tensor(cm[:], cm[:], ar[0:1, :], OP.max)
                        nc.gpsimd.partition_all_reduce(ar[0:10, :], qp[2][:], 10,
                                                       ReduceOp.max)
                        nc.vector.tensor_tensor(cm[:], cm[:], ar[0:1, :], OP.max)
                        # --- epsE = eps * e^{diag_q} * colmax
                        sqq = fp.tile([64, NPAD], BF16, tag='sqq', bufs=1, name='sqq')
                        nc.scalar.activation(sqq[:], qh[hh][:, base:base + NPAD],
                                             AF.Square, scale=DNS)
                        for b, bw in blocks(NPAD):
                            pq2 = ps1.tile([1, BLK], F32, tag='ps1', name='pq2')
                            nc.tensor.matmul(pq2[:, :bw], onesb[0:64, :],
                                             sqq[:, b:b + bw],
                                             start=True, stop=True)
                            nc.scalar.activation(epsE[:, b:b + bw], pq2[:, :bw],
                                                 AF.Exp, bias=cwp('clog', 1))
                        nc.vector.tensor_mul(epsE[:], epsE[:], cm[:])
                        # --- num_den [65, n] + eps rank-1; evict out into A
                        for b, bw in blocks(NPAD):
                            pnd = ps1.tile([65, BLK], F32, tag='ps1', name='pnd')
                            for ci, (m0, mw) in enumerate(MCH):
                                nc.tensor.matmul(pnd[:], ctx_sb[ci][0:mw, :],
                                                 qp[ci][0:mw, b:b + bw],
                                                 start=(ci == 0), stop=False)
                            nc.tensor.matmul(pnd[:], csr[:],
                                             epsE[:, b:b + bw],
                                             start=False, stop=True)
                            dinv = fp.tile([1, BLK], F32, tag='dinv', bufs=1,
                                           name='dinv')
                            nc.vector.reciprocal(dinv[:], pnd[64:65, :])
                            dvb = fp.tile([64, BLK], F32, tag='dvb', bufs=1,
                                          name='dvb')
                            nc.gpsimd.partition_broadcast(dvb[:], dinv[:], 64)
                            nc.vector.tensor_mul(
                                A[hp][hh * DH:(hh + 1) * DH,
                                      base + b:base + b + bw],
                                pnd[0:64, :], dvb[:])

            # ---- wo: S += A @ wo + bo
            for mc in range(DC):
                wt = wpp.tile([P, DC, P], BF16, tag='wpo', name='wto')
                nc.sync.dma_start(
                    wt[:], wob_d[l, :, mc * P:(mc + 1) * P]
                    .rearrange('(kc p) m -> p kc m', p=P))
                for b, bw in blocks(N2):
                    pm = pmm.tile([P, BLK], F32, tag='pmm', name='pmo')
                    for kc in range(DC):
                        nc.tensor.matmul(pm[:], wt[:, kc, :], A[kc][:, b:b + bw],
                                         start=(kc == 0), stop=(kc == DC - 1))
                    nc.vector.tensor_add(S[mc][:, b:b + bw], S[mc][:, b:b + bw],
                                         pm[:])
                nc.scalar.activation(S[mc][:], S[mc][:], AF.Identity,
                                     bias=cw(f'bo{l}', mc))

            # ---- FF in quarters of the 2048 hidden dim
            y2t = ln_fm(S, f'ln2w{l}', f'ln2b{l}', 0, N2, 'y')
            for q in range(4):
                w1q = fp.tile([P, DC, BLK], BF16, tag='w1q', bufs=1, name='w1q')
                nc.sync.dma_start(
                    w1q[:], f1_d[l, :, q * BLK:(q + 1) * BLK]
                    .rearrange('(kc p) m -> p kc m', p=P))
                w2q = fp.tile([P, DC, BLK], BF16, tag='w2q', bufs=1, name='w2q')
                nc.sync.dma_start(
                    w2q[:], f2_d[l, q * BLK:(q + 1) * BLK, :]
                    .rearrange('(kc p) m -> p kc m', p=P))
                for b, bw in blocks(N2):
                    H = []
                    for mc in range(DC):
                        pm = pmm.tile([P, BLK], F32, tag='pmm', name='pmf1')
                        for kc in range(DC):
                            nc.tensor.matmul(pm[:],
                                             w1q[:, kc, mc * P:(mc + 1) * P],
                                             y2t[kc][:, b:b + bw],
                                             start=(kc == 0), stop=(kc == DC - 1))
                        ht = hfp.tile([P, BLK], BF16, tag=f'H{mc}', name=f'H{mc}')
                        nc.scalar.activation(ht[:], pm[:], AF.Gelu,
                                             bias=cw(f'fb1{l}', q * 4 + mc))
                        H.append(ht)
                    for mc in range(DC):
                        pm = pmm.tile([P, BLK], F32, tag='pmm', name='pmf2')
                        for kc in range(DC):
                            nc.tensor.matmul(pm[:],
                                             w2q[:, kc, mc * P:(mc + 1) * P],
                                             H[kc][:],
                                             start=(kc == 0), stop=(kc == DC - 1))
                        nc.vector.tensor_add(S[mc][:, b:b + bw],
                                             S[mc][:, b:b + bw], pm[:])
            for mc in range(DC):
                nc.scalar.activation(S[mc][:], S[mc][:], AF.Identity,
                                     bias=cw(f'fb2{l}', mc))
                for s in range(2):
                    nc.vector.memset(S[mc][:, s * NPAD + 1 + L:(s + 1) * NPAD], 0.0)

        # -------------------------------------------------------- head
        clsx = [fp.tile([P, 2], F32, tag=f'cls{c}', name=f'cls{c}')
                for c in range(DC)]
        for c in range(DC):
            nc.vector.tensor_copy(clsx[c][:, 0:1], S[c][:, 0:1])
            nc.vector.tensor_copy(clsx[c][:, 1:2], S[c][:, NPAD:NPAD + 1])
        hx = ln_fm(clsx, 'hln_w', 'hln_b', 0, 2, 'y')
        hh_t = []
        for mc in range(DC):
            wt = wpp.tile([P, DC, P], BF16, tag='wpq', name='wth')
            nc.sync.dma_start(wt[:], hw1_d[:, mc * P:(mc + 1) * P]
                              .rearrange('(kc p) m -> p kc m', p=P))
            pm = ps1.tile([P, 2], F32, tag='ps1', name='pmh')
            for kc in range(DC):
                nc.tensor.matmul(pm[:], wt[:, kc, :], hx[kc][:, 0:2],
                                 start=(kc == 0), stop=(kc == DC - 1))
            ht = fp.tile([P, 2], F32, tag=f'hh{mc}', name=f'hhd{mc}')
            nc.scalar.activation(ht[:], pm[:], AF.Gelu, bias=cw('hb1', mc))
            hh_t.append(ht)
        wt2 = fp.tile([P, DC, 1], F32, tag='wt2', name='wt2')
        nc.sync.dma_start(wt2[:], hw2_d[:, :].rearrange('(kc p) m -> p kc m', p=P))
        po = ps1.tile([1, 2], F32, tag='ps1', name='po')
        for kc in range(DC):
            nc.tensor.matmul(po[:], wt2[:, kc, :], hh_t[kc][:, 0:2],
                             start=(kc == 0), stop=(kc == DC - 1))
        osb = fp.tile([1, 2], F32, tag='osb', name='osb')
        nc.scalar.activation(osb[:], po[:], AF.Identity,
                             bias=chw[0:1, COLS['hb2']:COLS['hb2'] + 1])
        nc.sync.dma_start(out_d[:], osb[:])

    return nc


# ---------------------------------------------------------------- host wrapper
def kernel(**inputs):
    inp = {k: np.asarray(v) for k, v in inputs.items()}
    B = inp['sig_n'].shape[0]
    assert B == 16, f'expected B=16, got {B}'

    sig = inp['sig_n'].astype(np.float32)
    x = np.where(np.isfinite(sig), sig, 0.0)
    x = np.where(x == -1.0, 0.0, x).astype(np.float32)
    valid = np.any(sig != -1.0, axis=1)                # [16, 1000]

    xh = np.zeros((B, 4, L + 8), np.float32)
    xh[:, :, 4:4 + L] = x
    t1full = np.zeros((B, 36, NPAD), np.float32)
    for t in range(9):
        t1full[:, 4 * t:4 * t + 4, 0:L] = xh[:, :, t:t + L]

    meta = inp['meta'].astype(np.int64)
    e_chr = inp['emb_chr'][np.clip(meta[:, 2], 0, 22)]
    e_gene = inp['emb_gene'][np.maximum(inp['gene_id'].astype(np.int64), 0)]
    e_exon = inp['emb_exon'][np.clip(inp['exon_id'].astype(np.int64), 0, 128)]
    e_ctx = (e_chr + e_gene + e_exon).astype(np.float32)

    pos = np.arange(L, dtype=np.float32)[:, None]
    div = np.exp(np.arange(0, D, 2, dtype=np.float32) * (-np.log(10000.0) / D))
    pe = np.zeros((L, D), np.float32)
    pe[:, 0::2] = np.sin(pos * div)
    pe[:, 1::2] = np.cos(pos * div)

    ADD = np.zeros((B, D, NPAD), np.float32)
    ADD[:, :, 0] = inp['cls'][0, 0][None, :] + e_ctx
    ADD[:, :, 1:1 + L] = pe.T[None] + e_ctx[:, :, None]

    w1t = np.zeros((36, 64), np.float32)
    for t in range(9):
        w1t[4 * t:4 * t + 4] = inp['conv1_w'][:, :, t].T
    w2t = np.ascontiguousarray(inp['conv2_w'].transpose(1, 2, 0)).astype(np.float32)
    w3t = np.ascontiguousarray(inp['conv3_w'][:, :, 0].T).astype(np.float32)
    gmt = np.zeros((8, P), np.float32)
    for p in range(P):
        gmt[p // 16, p] = 1.0
    projt = np.ascontiguousarray((inp['proj'] * DN).transpose(0, 2, 1)).astype(np.float32)

    bf = lambda a: np.ascontiguousarray(np.asarray(a, np.float32).astype(ml_dtypes.bfloat16))
    shared = dict(
        w1t=w1t, w2t=w2t, w3t=w3t, gmt=gmt, projt=projt,
        wqb=bf(inp['wq']), wkb=bf(inp['wk']), wvb=bf(inp['wv']),
        wob=bf(inp['wo']),
        f1b=bf(inp['ff_w1']), f2b=bf(inp['ff_w2']),
        hw1b=bf(inp['h_w1']),
        hw2=np.ascontiguousarray(inp['h_w2'], dtype=np.float32),
    )

    in_maps = []
    for c in range(N_CORES):
        b0 = 2 * c
        real01 = np.zeros((P, 16), np.float32)
        vm01 = np.zeros((P, 16), np.float32)
        for s in range(2):
            for n in range(NPAD):
                t, row = n // P, n % P
                if n <= L:
                    real01[row, s * 8 + t] = 1.0
                    if n == 0 or valid[b0 + s, n - 1]:
                        vm01[row, s * 8 + t] = 1.0
        chw = pack_chw(inp, real01, vm01)
        in_maps.append(dict(
            shared,
            xh=np.ascontiguousarray(t1full[b0:b0 + 2]),
            addt=np.ascontiguousarray(ADD[b0:b0 + 2]),
            chw=chw,
        ))

    nc = bacc.Bacc()
    build(nc)
    nc.finalize()
    res = run_bass_kernel_spmd(nc, in_maps, list(range(N_CORES)))
    global LAST_RESULT
    LAST_RESULT = res
    out = np.concatenate([np.asarray(res.results[c]['o']).reshape(2)
                          for c in range(N_CORES)])
    return out.astype(np.float32)


LAST_RESULT = None


if __name__ == '__main__':
    import reference
    inputs = {k: np.asarray(v) for k, v in reference.setup_inputs().items()}
    got = kernel(**inputs)
    print('kernel out:', got)



# revision 5
# speedup vs baseline: 1.1009x; 1.1009x over previous
"""Trainium2 Bass kernel for nn_CNVRegressor (CNN tokenizer + 5-layer Performer + head).

Sharding: data-parallel over batch B=16 across 8 cores (2 samples/core).
Layout: feature-major activations [D on partitions, tokens on free].
Per-sample sequence padded 1001 -> 1024; two samples side by side -> [512, 2048].

Single-pass FAVOR+: kp = exp(ddk - diagk) is computed unstabilized (safe in
f32/bf16 since |ddk| <~ 10); the reference's global key-stab enters ONLY via
the eps-term coefficient gamma = eps*exp(stab_g). The per-core max is taken as
a cheap byproduct of the kp tiles (DVE max + log), AllReduce(max)'d across the
8 cores while the q-side computes, then folded into the rank-1 eps correction.
This removes the old dedicated k-stab pass (a full K projection + 266-wide
scan per layer) with no numerical change.

Self-contained: hardcodes all shapes; host does only input prep / sharding
(cleanup, halo pad, embedding row gather, PE table, bias/mask packing).
"""
import math
from contextlib import ExitStack

import ml_dtypes
import numpy as np

import concourse.bass as bass
import concourse.bacc as bacc
import concourse.tile as tile
from concourse import mybir
from concourse.bass_isa import ReduceOp
from concourse.bass_utils import run_bass_kernel_spmd
from concourse.masks import make_identity

F32 = mybir.dt.float32
F32R = mybir.dt.float32r
BF16 = mybir.dt.bfloat16
AF = mybir.ActivationFunctionType
OP = mybir.AluOpType
AX = mybir.AxisListType

P = 128
D = 512
DH = 64
DEPTH = 5
L = 1000
M = 266
NPAD = 1024
N2 = 2 * NPAD
NT = NPAD // P            # 8 token tiles per sample
DC = D // P               # 4 feature chunks
BLK = 512
DN = DH ** -0.25
DNS = DN * math.sqrt(0.5)
LN_EPS = 1e-5
GN_EPS = 1e-5
LOG_EPS = float(np.log(1e-4))
MCH = ((0, 128), (128, 128), (256, 10))   # m-chunks of 266
N_CORES = 8


def build_cols():
    cols, idx = {}, 0

    def a(name, n):
        nonlocal idx
        cols[name] = idx
        idx += n

    for n in ('gn_w', 'gn_b', 'lnt_w', 'lnt_b'):
        a(n, 4)
    a('b1', 1); a('b2', 1); a('b3', 4); a('gmask', 8)
    a('real', 16); a('stabb', 16); a('vmask', 16)
    for n in ('hln_w', 'hln_b', 'hb1'):
        a(n, 4)
    a('hb2', 1)
    a('cln', 1)
    a('clog', 1)
    a('ctiny', 1)
    for l in range(DEPTH):
        for n in ('ln1w', 'ln1b', 'ln2w', 'ln2b', 'bq', 'bk', 'bv', 'bo', 'fb2'):
            a(f'{n}{l}', 4)
        a(f'fb1{l}', 16)
    return cols, idx


COLS, NCOL = build_cols()


def pack_chw(inp, real01, vmask01):
    chw = np.zeros((P, NCOL), np.float32)

    def put(name, vec):
        vec = np.asarray(vec, np.float32).reshape(-1)
        c0 = COLS[name]
        for c in range((len(vec) + P - 1) // P):
            seg = vec[c * P:(c + 1) * P]
            chw[:len(seg), c0 + c] = seg

    put('gn_w', inp['gn_w']); put('gn_b', inp['gn_b'])
    put('lnt_w', inp['lnt_w']); put('lnt_b', inp['lnt_b'])
    put('b1', inp['conv1_b']); put('b2', inp['conv2_b']); put('b3', inp['conv3_b'])
    gm = np.zeros((P, 8), np.float32)
    for p in range(P):
        gm[p, p // 16] = 1.0
    chw[:, COLS['gmask']:COLS['gmask'] + 8] = gm
    chw[:, COLS['real']:COLS['real'] + 16] = real01
    chw[:, COLS['stabb']:COLS['stabb'] + 16] = (real01 - 1.0) * 1e30
    chw[:, COLS['vmask']:COLS['vmask'] + 16] = vmask01
    put('hln_w', inp['h_ln_w']); put('hln_b', inp['h_ln_b'])
    put('hb1', inp['h_b1']); put('hb2', inp['h_b2'])
    chw[:, COLS['cln']] = LN_EPS
    chw[:, COLS['clog']] = LOG_EPS
    chw[:, COLS['ctiny']] = 1e-30
    for l in range(DEPTH):
        put(f'ln1w{l}', inp['ln1_w'][l]); put(f'ln1b{l}', inp['ln1_b'][l])
        put(f'ln2w{l}', inp['ln2_w'][l]); put(f'ln2b{l}', inp['ln2_b'][l])
        put(f'bq{l}', inp['bq'][l]); put(f'bk{l}', inp['bk'][l])
        put(f'bv{l}', inp['bv'][l]); put(f'bo{l}', inp['bo'][l])
        put(f'fb1{l}', inp['ff_b1'][l]); put(f'fb2{l}', inp['ff_b2'][l])
    return chw


def blocks(width, bs=BLK, off0=0):
    out, off = [], 0
    while off < width:
        out.append((off0 + off, min(bs, width - off)))
        off += bs
    return out


# ---------------------------------------------------------------- device build
def build(nc):
    dp = lambda n, sh, dt=F32: nc.declare_dram_parameter(n, sh, dt, isOutput=False)
    xh_d = dp('xh', (2, 36, NPAD))
    add_d = dp('addt', (2, D, NPAD))
    w1t_d = dp('w1t', (36, 64))
    w2t_d = dp('w2t', (64, 9, 128))
    w3t_d = dp('w3t', (128, D))
    chw_d = dp('chw', (P, NCOL))
    gmt_d = dp('gmt', (8, P))
    proj_d = dp('projt', (DEPTH, DH, M))
    wq_d = dp('wqb', (DEPTH, D, D), BF16)
    wk_d = dp('wkb', (DEPTH, D, D), BF16)
    wv_d = dp('wvb', (DEPTH, D, D), BF16)
    wob_d = dp('wob', (DEPTH, D, D), BF16)
    f1_d = dp('f1b', (DEPTH, D, 4 * D), BF16)
    f2_d = dp('f2b', (DEPTH, 4 * D, D), BF16)
    hw1_d = dp('hw1b', (D, D), BF16)
    hw2_d = dp('hw2', (D, 1))
    out_d = nc.declare_dram_parameter('o', (1, 2), F32, isOutput=True)

    with tile.TileContext(nc) as tc, ExitStack() as ctx:
        const = ctx.enter_context(tc.tile_pool(name='const', bufs=1))
        sp = ctx.enter_context(tc.tile_pool(name='sp', bufs=1))
        actp = ctx.enter_context(tc.tile_pool(name='actp', bufs=1))
        wpp = ctx.enter_context(tc.tile_pool(name='wpp', bufs=2))
        fp = ctx.enter_context(tc.tile_pool(name='fp', bufs=2))
        kvp = ctx.enter_context(tc.tile_pool(name='kvp', bufs=3))
        dram = ctx.enter_context(tc.tile_pool(name='dram', bufs=2, space='DRAM'))
        pmm = ctx.enter_context(tc.tile_pool(name='pmm', bufs=4, space='PSUM'))
        ps1 = ctx.enter_context(tc.tile_pool(name='ps1', bufs=3, space='PSUM'))

        # ---- constants
        chw = const.tile([P, NCOL], F32, name='chw')
        nc.sync.dma_start(chw[:], chw_d[:])
        cw = lambda name, off=0: chw[:, COLS[name] + off:COLS[name] + off + 1]
        cwp = lambda name, parts: chw[0:parts, COLS[name]:COLS[name] + 1]
        ident = const.tile([P, P], F32, name='ident')
        make_identity(nc, ident)
        identb = const.tile([P, P], BF16, name='identb')
        nc.vector.tensor_copy(identb[:], ident[:])
        ones = const.tile([P, 1], F32, name='ones')
        nc.vector.memset(ones[:], 1.0)
        onesb = const.tile([P, 1], BF16, name='onesb')
        nc.vector.memset(onesb[:], 1.0)
        gmt = const.tile([8, P], F32, name='gmt')
        nc.sync.dma_start(gmt[:], gmt_d[:])
        w1t = const.tile([36, 64], F32, name='w1t')
        nc.sync.dma_start(w1t[:], w1t_d[:])
        w2t = const.tile([64, 9, 128], F32, name='w2t')
        nc.sync.dma_start(w2t[:], w2t_d[:])
        w3t = const.tile([128, D], F32, name='w3t')
        nc.sync.dma_start(w3t[:], w3t_d[:])

        S = [sp.tile([P, N2], F32, tag=f'S{c}', name=f'S{c}') for c in range(DC)]

        # -------------------------------------------------------- shared LN
        def ln_fm(X, wc, bc, col0, width, ytag):
            """Per-token LN over the 512 partition dim (feature-major).

            X tiles are f32; returns bf16 normed tiles."""
            Y = [actp.tile([P, N2], BF16, tag=f'{ytag}{c}', name=f'{ytag}{c}')
                 for c in range(DC)]
            for c in range(DC):
                nc.vector.tensor_mul(Y[c][:, col0:col0 + width],
                                     X[c][:, col0:col0 + width],
                                     X[c][:, col0:col0 + width])
            srow = fp.tile([1, N2], F32, tag='srow', bufs=1, name='srow')
            qrow = fp.tile([1, N2], F32, tag='qrow', bufs=1, name='qrow')
            trow = fp.tile([1, N2], F32, tag='trow', bufs=1, name='trow')
            mrow, vrow = srow, qrow
            for o, bw in blocks(width, BLK, col0):
                ps = ps1.tile([1, BLK], F32, tag='ps1', name='ps')
                pq = ps1.tile([1, BLK], F32, tag='ps1', name='pq')
                for c in range(DC):
                    nc.tensor.matmul(ps[:, :bw], ones[:], X[c][:, o:o + bw],
                                     start=(c == 0), stop=(c == DC - 1))
                    nc.tensor.matmul(pq[:, :bw], onesb[:], Y[c][:, o:o + bw],
                                     start=(c == 0), stop=(c == DC - 1))
                nc.any.tensor_copy(srow[:, o:o + bw], ps[:, :bw])
                nc.any.tensor_copy(qrow[:, o:o + bw], pq[:, :bw])
            sl = slice(col0, col0 + width)
            nc.vector.tensor_scalar_mul(mrow[:, sl], srow[:, sl], 1.0 / D)
            nc.vector.tensor_mul(trow[:, sl], mrow[:, sl], mrow[:, sl])
            nc.vector.tensor_scalar(vrow[:, sl], qrow[:, sl], 1.0 / D, None,
                                    OP.mult)
            nc.vector.tensor_sub(vrow[:, sl], vrow[:, sl], trow[:, sl])
            nc.scalar.activation(vrow[:, sl], vrow[:, sl], AF.Ln,
                                 bias=cwp('cln', 1))
            nc.scalar.activation(vrow[:, sl], vrow[:, sl], AF.Exp, scale=-0.5)
            MU = actp.tile([P, N2], F32, tag='MU', name='MU')
            RS = actp.tile([P, N2], F32, tag='RS', name='RS')
            nc.gpsimd.partition_broadcast(MU[:, sl], mrow[:, sl], P)
            nc.gpsimd.partition_broadcast(RS[:, sl], vrow[:, sl], P)
            for c in range(DC):
                nc.vector.tensor_sub(Y[c][:, sl], X[c][:, sl], MU[:, sl])
                nc.vector.tensor_mul(Y[c][:, sl], Y[c][:, sl], RS[:, sl])
                nc.scalar.activation(Y[c][:, sl], Y[c][:, sl], AF.Identity,
                                     scale=cw(wc, c), bias=cw(bc, c))
            return Y

        # full-width projection: dst[128, N2] = (w^T y) + bias, both heads of
        # a pair. kc-outer so each stationary is loaded once per 4 blocks.
        def proj_mm(wd, l, hp, bn, dst):
            wt = wpp.tile([P, DC, P], BF16, tag='wpq', name='wt')
            nc.sync.dma_start(
                wt[:], wd[l, :, hp * P:(hp + 1) * P]
                .rearrange('(kc p) m -> p kc m', p=P))
            bias = chw[:, COLS[f'{bn}{l}'] + hp:COLS[f'{bn}{l}'] + hp + 1]
            pms = [pmm.tile([P, BLK], F32, tag='pmm', name=f'pm{bi}')
                   for bi in range(4)]
            for kc in range(DC):
                for bi, (b, bw) in enumerate(blocks(N2)):
                    nc.tensor.matmul(pms[bi][:], wt[:, kc, :], y[kc][:, b:b + bw],
                                     start=(kc == 0), stop=(kc == DC - 1))
            for bi, (b, bw) in enumerate(blocks(N2)):
                nc.scalar.activation(dst[:, b:b + bw], pms[bi][:], AF.Identity,
                                     bias=bias)

        # -------------------------------------------------------- tokenizer
        for s in range(2):
            for c in range(DC):
                nc.sync.dma_start(S[c][:, s * NPAD:(s + 1) * NPAD],
                                  add_d[s, c * P:(c + 1) * P, :])
        for s in range(2):
            t1 = fp.tile([36, NPAD], F32, tag='tokA', bufs=1, name='t1')
            nc.sync.dma_start(t1[:], xh_d[s])
            y1h = fp.tile([64, L + 8], F32, tag='tokB', bufs=1, name='y1h')
            nc.vector.memset(y1h[:], 0.0)
            for o, bw in blocks(L):
                p1 = pmm.tile([64, BLK], F32, tag='pmm', name='p1')
                nc.tensor.matmul(p1[:, :bw], w1t[:], t1[:, o:o + bw],
                                 start=True, stop=True)
                nc.scalar.activation(y1h[:, 4 + o:4 + o + bw], p1[:, :bw], AF.Gelu,
                                     bias=chw[0:64, COLS['b1']:COLS['b1'] + 1])
            y2 = fp.tile([P, NPAD], F32, tag='tokA', bufs=1, name='y2')
            for o, bw in blocks(L):
                p2 = pmm.tile([P, BLK], F32, tag='pmm', name='p2')
                for t in range(9):
                    nc.tensor.matmul(p2[:, :bw], w2t[:, t, :],
                                     y1h[:, t + o:t + o + bw],
                                     start=(t == 0), stop=(t == 8))
                nc.scalar.activation(y2[:, o:o + bw], p2[:, :bw], AF.Gelu,
                                     bias=cw('b2'))
            x3 = [actp.tile([P, NPAD], F32, tag=f'A{c}', name=f'x3{c}')
                  for c in range(DC)]
            for c in range(DC):
                for o, bw in blocks(L):
                    p3 = pmm.tile([P, BLK], F32, tag='pmm', name='p3')
                    nc.tensor.matmul(p3[:, :bw], w3t[:, c * P:(c + 1) * P],
                                     y2[:, o:o + bw], start=True, stop=True)
                    nc.scalar.activation(x3[c][:, o:o + bw], p3[:, :bw],
                                         AF.Identity, bias=cw('b3', c))
            # GroupNorm(32, 512) over [16ch x 1000]
            stats = fp.tile([P, 8], F32, tag='gstats', name='stats')
            sqt = fp.tile([P, NPAD], F32, tag='tokB', bufs=1, name='sqt')
            for c in range(DC):
                nc.vector.tensor_reduce(stats[:, c:c + 1], x3[c][:, 0:L], AX.X, OP.add)
                nc.vector.tensor_mul(sqt[:, 0:L], x3[c][:, 0:L], x3[c][:, 0:L])
                nc.vector.tensor_reduce(stats[:, 4 + c:5 + c], sqt[:, 0:L], AX.X, OP.add)
            pg = ps1.tile([8, 8], F32, tag='ps1', name='pg')
            nc.tensor.matmul(pg[:], chw[:, COLS['gmask']:COLS['gmask'] + 8],
                             stats[:], start=True, stop=True)
            gs = fp.tile([8, 8], F32, tag='gs', name='gs')
            nc.vector.tensor_scalar_mul(gs[:], pg[:], 1.0 / (16 * L))
            gm2 = fp.tile([8, 4], F32, tag='gm2', name='gm2')
            nc.vector.tensor_mul(gm2[:], gs[:, 0:4], gs[:, 0:4])
            nc.vector.tensor_sub(gs[:, 4:8], gs[:, 4:8], gm2[:])
            nc.scalar.activation(gs[:, 4:8], gs[:, 4:8], AF.Ln, bias=cwp('cln', 8))
            nc.scalar.activation(gs[:, 4:8], gs[:, 4:8], AF.Exp, scale=-0.5)
            pb = ps1.tile([P, 8], F32, tag='ps1', name='pb')
            nc.tensor.matmul(pb[:], gmt[:], gs[:], start=True, stop=True)
            cstat = fp.tile([P, 8], F32, tag='cstat', name='cstat')
            nc.vector.tensor_copy(cstat[:], pb[:])
            for c in range(DC):
                nc.vector.tensor_scalar(x3[c][:, 0:L], x3[c][:, 0:L],
                                        cstat[:, c:c + 1], cstat[:, 4 + c:5 + c],
                                        OP.subtract, OP.mult)
                nc.scalar.activation(x3[c][:, 0:L], x3[c][:, 0:L], AF.Identity,
                                     scale=cw('gn_w', c), bias=cw('gn_b', c))
            tok = ln_fm(x3, 'lnt_w', 'lnt_b', 0, L, 'y')
            b0 = s * NPAD
            for c in range(DC):
                nc.vector.tensor_add(S[c][:, b0 + 1:b0 + 1 + L],
                                     S[c][:, b0 + 1:b0 + 1 + L], tok[c][:, 0:L])

        # -------------------------------------------------------- layers
        for l in range(DEPTH):
            projT = fp.tile([DH, M], F32, tag='projT', name='projT')
            nc.sync.dma_start(projT[:], proj_d[l])
            # duplicated into both partition halves so head-1 slices
            # (base partition 64) can pair with it in matmuls; the second
            # half is filled by DMA (engines can't shift partitions)
            projTb = fp.tile([P, M], BF16, tag='projTb', name='projTb')
            nc.vector.tensor_copy(projTb[0:DH, :], projT[:])
            nc.sync.dma_start(projTb[DH:P, :], projTb[0:DH, :])

            y = ln_fm(S, f'ln1w{l}', f'ln1b{l}', 0, N2, 'y')

            # per-layer k-side context accumulators [65, 272] x 16 head-samples
            ctxE = fp.tile([65, 16, 272], BF16, tag='ctxE', bufs=1, name='ctxE')
            smax = fp.tile([P, 16], F32, tag='smax', name='smax')

            # ---- phase K: k/v projections, kp, ctx/ksum, local stab byproduct
            for hp in range(4):
                kh2 = fp.tile([P, N2], BF16, tag='kh2', bufs=2, name='kh2')
                vh2 = fp.tile([P, N2], BF16, tag='vh2', bufs=2, name='vh2')
                proj_mm(wk_d, l, hp, 'bk', kh2)
                proj_mm(wv_d, l, hp, 'bv', vh2)
                for hh in range(2):
                    hsl = slice(hh * DH, (hh + 1) * DH)
                    for s in range(2):
                        base = s * NPAD
                        idx = hp * 4 + hh * 2 + s
                        # --- -diag_k per token, [P, 8]; pad rows get -1e30
                        biask = fp.tile([P, 8], F32, tag='biask', name='biask')
                        for t in range(NT):
                            csl = slice(base + t * P, base + (t + 1) * P)
                            pkt = ps1.tile([P, 64], BF16, tag='ps1', name='pkt')
                            nc.tensor.transpose(pkt[:], kh2[hsl, csl],
                                                identb[hsl, hsl])
                            sqk = fp.tile([P, 64], F32, tag='sqk', name='sqk')
                            nc.scalar.activation(sqk[:], pkt[:], AF.Square,
                                                 scale=DNS)
                            nc.vector.tensor_reduce(biask[:, t:t + 1], sqk[:],
                                                    AX.X, OP.add)
                        nc.vector.tensor_scalar_mul(biask[:], biask[:], -1.0)
                        bmask = fp.tile([P, 8], F32, tag='bmask', name='bmask')
                        nc.vector.tensor_add(
                            bmask[:], biask[:],
                            chw[:, COLS['stabb'] + s * 8:COLS['stabb'] + s * 8 + 8])
                        # --- kp tiles + ctx'^T [65, 267] accumulation
                        pctx = ps1.tile([65, 272], F32, tag='psx', bufs=1,
                                        name='pctx')
                        rm8 = fp.tile([P, 8], F32, tag='rm8', name='rm8')
                        for t in range(NT):
                            csl = slice(base + t * P, base + (t + 1) * P)
                            pdk = ps1.tile([P, 272], F32, tag='ps1', name='pdk')
                            nc.tensor.matmul(pdk[:, 0:M], kh2[hsl, csl],
                                             projTb[hsl, :], start=True, stop=True)
                            kp = kvp.tile([P, 272], BF16, tag='kp', name='kp')
                            nc.scalar.activation(kp[:, 0:M], pdk[:, 0:M], AF.Exp,
                                                 bias=bmask[:, t:t + 1])
                            nc.vector.tensor_copy(kp[:, M:M + 1],
                                                  cw('real', s * 8 + t))
                            nc.vector.tensor_reduce(rm8[:, t:t + 1], kp[:, 0:M],
                                                    AX.X, OP.max)
                            pvt = ps1.tile([P, 64], BF16, tag='ps1', name='pvt')
                            nc.tensor.transpose(pvt[:], vh2[hsl, csl],
                                                identb[hsl, hsl])
                            v1 = kvp.tile([P, 65], BF16, tag='v1', name='v1')
                            nc.vector.tensor_scalar_mul(v1[:, 0:64], pvt[:],
                                                        cw('vmask', s * 8 + t))
                            nc.vector.memset(v1[:, 64:65], 1.0)
                            nc.tensor.matmul(pctx[:, 0:M + 1], v1[:], kp[:, 0:M + 1],
                                             start=(t == 0), stop=(t == NT - 1))
                        nc.vector.tensor_copy(ctxE[:, idx, 0:M + 1],
                                              pctx[:, 0:M + 1])
                        # --- local stab byproduct: max_t(ln(max_m kp) + diag)
                        m8 = fp.tile([P, 8], F32, tag='m8', name='m8')
                        nc.scalar.activation(m8[:], rm8[:], AF.Ln,
                                             bias=cw('ctiny'))
                        nc.vector.tensor_sub(m8[:], m8[:], biask[:])
                        nc.vector.tensor_reduce(smax[:, idx:idx + 1], m8[:],
                                                AX.X, OP.max)

            # ---- global key-stab: AllReduce(max) across the 8 cores.
            # Overlaps with the q-side below (consumed only at ctxT/vsc).
            sfin = fp.tile([P, 1], F32, tag='sfin', name='sfin')
            nc.vector.tensor_reduce(sfin[:], smax[:], AX.X, OP.max)
            nc.gpsimd.partition_all_reduce(sfin[:], sfin[:], P, ReduceOp.max)
            bin_ = dram.tile([P, 1], F32, name='bin')
            bout = dram.tile([P, 1], F32, name='bout')
            nc.sync.dma_start(bin_[:], sfin[:])
            nc.gpsimd.collective_compute(
                'AllReduce', OP.max,
                replica_groups=[list(range(N_CORES))],
                ins=[bin_.opt()], outs=[bout.opt()])
            stabg = fp.tile([P, 1], F32, tag='stabg', name='stabg')
            nc.sync.dma_start(stabg[:], bout[:])
            ceps65 = fp.tile([65, 1], F32, tag='ceps65', name='ceps65')
            nc.scalar.activation(ceps65[:], stabg[0:65, :], AF.Exp,
                                 bias=cwp('clog', 65))

            # ---- phase Q: q projection, qp, eps corrections, num/den -> A
            A = [actp.tile([P, N2], BF16, tag=f'A{c}', name=f'Aa{c}')
                 for c in range(DC)]
            for hp in range(4):
                qh2 = fp.tile([P, N2], BF16, tag='qh2', bufs=2, name='qh2')
                proj_mm(wq_d, l, hp, 'bq', qh2)
                # (dn^2/2)*q^2 for diag_q, both heads/samples in one op
                sqq = fp.tile([P, N2], BF16, tag='sqq', bufs=1, name='sqq')
                nc.scalar.activation(sqq[:], qh2[:], AF.Square, scale=DNS)
                for hh in range(2):
                    hsl = slice(hh * DH, (hh + 1) * DH)
                    for s in range(2):
                        base = s * NPAD
                        idx = hp * 4 + hh * 2 + s
                        # --- qp = exp(ddq), feature-major
                        qp = [fp.tile([P, NPAD], BF16, tag='qp0', bufs=1, name='qp0'),
                              fp.tile([P, NPAD], BF16, tag='qp1', bufs=1, name='qp1'),
                              fp.tile([10, NPAD], BF16, tag='qp2', bufs=1, name='qp2')]
                        for ci, (m0, mw) in enumerate(MCH):
                            for b, bw in blocks(NPAD):
                                pdq = pmm.tile([P, BLK], F32, tag='pmm', name='pdq')
                                nc.tensor.matmul(
                                    pdq[0:mw, :], projTb[hsl, m0:m0 + mw],
                                    qh2[hsl, base + b:base + b + bw],
                                    start=True, stop=True)
                                nc.scalar.activation(qp[ci][0:mw, b:b + bw],
                                                     pdq[0:mw, :], AF.Exp)
                        # --- colmax = e^{stab_q} per token: fold the 3 chunks
                        # with same-partition DVE maxes, then ONE gpsimd
                        # all-reduce (was three)
                        ar = fp.tile([P, NPAD], BF16, tag='ar', bufs=1, name='ar')
                        nc.vector.tensor_tensor(ar[:], qp[0][:], qp[1][:], OP.max)
                        nc.vector.tensor_tensor(ar[0:10, :], ar[0:10, :],
                                                qp[2][:], OP.max)
                        nc.gpsimd.partition_all_reduce(ar[:], ar[:], P,
                                                       ReduceOp.max)
                        # --- epsE = eps * e^{diag_q} * colmax
                        epsE = fp.tile([1, NPAD], BF16, tag='epsE', bufs=1,
                                       name='epsE')
                        for b, bw in blocks(NPAD):
                            pq2 = ps1.tile([1, BLK], F32, tag='ps1', name='pq2')
                            nc.tensor.matmul(pq2[:, :bw], onesb[hsl, :],
                                             sqq[hsl, base + b:base + b + bw],
                                             start=True, stop=True)
                            nc.scalar.activation(epsE[:, b:b + bw], pq2[:, :bw],
                                                 AF.Exp, bias=cwp('clog', 1))
                        nc.vector.tensor_mul(epsE[:], epsE[:], ar[0:1, :])
                        # --- ctxT = ctxE + gamma * vsum; -> [266(+1), 65] chunks
                        vsc = fp.tile([65, 1], F32, tag='vsc', name='vsc')
                        nc.vector.tensor_mul(vsc[:], ctxE[:, idx, M:M + 1],
                                             ceps65[:])
                        ctxT = fp.tile([65, M], BF16, tag='ctxT', name='ctxT')
                        nc.vector.tensor_scalar(ctxT[:], ctxE[:, idx, 0:M],
                                                vsc[:], None, OP.add)
                        ctx_sb = []
                        for ci, (m0, mw) in enumerate(MCH):
                            ptc = ps1.tile([P, 65], BF16, tag='ps1', name='ptc')
                            nc.tensor.transpose(ptc[0:mw, :], ctxT[:, m0:m0 + mw],
                                                identb[0:65, 0:65])
                            csb = fp.tile([P, 65], BF16, tag=f'ctx{ci}', name=f'c{ci}')
                            nc.any.tensor_copy(csb[0:mw, :], ptc[0:mw, :])
                            ctx_sb.append(csb)
                        # csr = colsum of ctxT, for the rank-1 eps term
                        pcs = ps1.tile([1, 65], F32, tag='ps1', name='pcs')
                        for ci, (m0, mw) in enumerate(MCH):
                            nc.tensor.matmul(pcs[:], onesb[0:mw, :],
                                             ctx_sb[ci][0:mw, :],
                                             start=(ci == 0), stop=(ci == 2))
                        csr = fp.tile([1, 65], BF16, tag='csr', name='csr')
                        nc.any.tensor_copy(csr[:], pcs[:])
                        # --- num_den [65, n]; rows 0..63 num, row 64 den
                        for b, bw in blocks(NPAD):
                            pnd = ps1.tile([65, BLK], F32, tag='ps1', name='pnd')
                            for ci, (m0, mw) in enumerate(MCH):
                                nc.tensor.matmul(pnd[:], ctx_sb[ci][0:mw, :],
                                                 qp[ci][0:mw, b:b + bw],
                                                 start=(ci == 0), stop=False)
                            nc.tensor.matmul(pnd[:], csr[:], epsE[:, b:b + bw],
                                             start=False, stop=True)
                            dinv = fp.tile([1, BLK], F32, tag='dinv', bufs=1,
                                           name='dinv')
                            nc.scalar.activation(dinv[:], pnd[64:65, :], AF.Ln)
                            nc.scalar.activation(dinv[:], dinv[:], AF.Exp,
                                                 scale=-1.0)
                            dvb = fp.tile([64, BLK], F32, tag='dvb', bufs=1,
                                          name='dvb')
                            nc.gpsimd.partition_broadcast(dvb[:], dinv[:], 64)
                            nc.vector.tensor_mul(
                                A[hp][hsl, base + b:base + b + bw],
                                pnd[0:64, :], dvb[:])

            # ---- wo: S += A @ wo + bo (kc-outer for stationary reuse)
            for mc in range(DC):
                wt = wpp.tile([P, DC, P], BF16, tag='wpo', name='wto')
                nc.sync.dma_start(
                    wt[:], wob_d[l, :, mc * P:(mc + 1) * P]
                    .rearrange('(kc p) m -> p kc m', p=P))
                pms = [pmm.tile([P, BLK], F32, tag='pmm', name=f'pmo{bi}')
                       for bi in range(4)]
                for kc in range(DC):
                    for bi, (b, bw) in enumerate(blocks(N2)):
                        nc.tensor.matmul(pms[bi][:], wt[:, kc, :],
                                         A[kc][:, b:b + bw],
                                         start=(kc == 0), stop=(kc == DC - 1))
                for bi, (b, bw) in enumerate(blocks(N2)):
                    nc.vector.tensor_add(S[mc][:, b:b + bw], S[mc][:, b:b + bw],
                                         pms[bi][:])
                nc.scalar.activation(S[mc][:], S[mc][:], AF.Identity,
                                     bias=cw(f'bo{l}', mc))

            # ---- FF in quarters of the 2048 hidden dim (kc-outer)
            y2t = ln_fm(S, f'ln2w{l}', f'ln2b{l}', 0, N2, 'y')
            for q in range(4):
                w1q = fp.tile([P, DC, BLK], BF16, tag='w1q', bufs=1, name='w1q')
                nc.sync.dma_start(
                    w1q[:], f1_d[l, :, q * BLK:(q + 1) * BLK]
                    .rearrange('(kc p) m -> p kc m', p=P))
                w2q = fp.tile([P, DC, BLK], BF16, tag='w2q', bufs=1, name='w2q')
                nc.sync.dma_start(
                    w2q[:], f2_d[l, q * BLK:(q + 1) * BLK, :]
                    .rearrange('(kc p) m -> p kc m', p=P))
                # reuse the dead A tiles (same shape) for the FF hidden
                H = [actp.tile([P, N2], BF16, tag=f'A{mc}', name=f'H{mc}')
                     for mc in range(DC)]
                for mc in range(DC):
                    pms = [pmm.tile([P, BLK], F32, tag='pmm', name=f'pmf1{bi}')
                           for bi in range(4)]
                    for kc in range(DC):
                        for bi, (b, bw) in enumerate(blocks(N2)):
                            nc.tensor.matmul(pms[bi][:],
                                             w1q[:, kc, mc * P:(mc + 1) * P],
                                             y2t[kc][:, b:b + bw],
                                             start=(kc == 0), stop=(kc == DC - 1))
                    for bi, (b, bw) in enumerate(blocks(N2)):
                        nc.scalar.activation(H[mc][:, b:b + bw], pms[bi][:],
                                             AF.Gelu, bias=cw(f'fb1{l}', q * 4 + mc))
                for mc in range(DC):
                    pms = [pmm.tile([P, BLK], F32, tag='pmm', name=f'pmf2{bi}')
                           for bi in range(4)]
                    for kc in range(DC):
                        for bi, (b, bw) in enumerate(blocks(N2)):
                            nc.tensor.matmul(pms[bi][:],
                                             w2q[:, kc, mc * P:(mc + 1) * P],
                                             H[kc][:, b:b + bw],
                                             start=(kc == 0), stop=(kc == DC - 1))
                    for bi, (b, bw) in enumerate(blocks(N2)):
                        nc.vector.tensor_add(S[mc][:, b:b + bw],
                                             S[mc][:, b:b + bw], pms[bi][:])
            for mc in range(DC):
                nc.scalar.activation(S[mc][:], S[mc][:], AF.Identity,
                                     bias=cw(f'fb2{l}', mc))
                for s in range(2):
                    nc.vector.memset(S[mc][:, s * NPAD + 1 + L:(s + 1) * NPAD], 0.0)

        # -------------------------------------------------------- head
        clsx = [fp.tile([P, 2], F32, tag=f'cls{c}', name=f'cls{c}')
                for c in range(DC)]
        for c in range(DC):
            nc.vector.tensor_copy(clsx[c][:, 0:1], S[c][:, 0:1])
            nc.vector.tensor_copy(clsx[c][:, 1:2], S[c][:, NPAD:NPAD + 1])
        hx = ln_fm(clsx, 'hln_w', 'hln_b', 0, 2, 'y')
        hh_t = []
        for mc in range(DC):
            wt = wpp.tile([P, DC, P], BF16, tag='wpq', name='wth')
            nc.sync.dma_start(wt[:], hw1_d[:, mc * P:(mc + 1) * P]
                              .rearrange('(kc p) m -> p kc m', p=P))
            pm = ps1.tile([P, 2], F32, tag='ps1', name='pmh')
            for kc in range(DC):
                nc.tensor.matmul(pm[:], wt[:, kc, :], hx[kc][:, 0:2],
                                 start=(kc == 0), stop=(kc == DC - 1))
            ht = fp.tile([P, 2], F32, tag=f'hh{mc}', name=f'hhd{mc}')
            nc.scalar.activation(ht[:], pm[:], AF.Gelu, bias=cw('hb1', mc))
            hh_t.append(ht)
        wt2 = fp.tile([P, DC, 1], F32, tag='wt2', name='wt2')
        nc.sync.dma_start(wt2[:], hw2_d[:, :].rearrange('(kc p) m -> p kc m', p=P))
        po = ps1.tile([1, 2], F32, tag='ps1', name='po')
        for kc in range(DC):
            nc.tensor.matmul(po[:], wt2[:, kc, :], hh_t[kc][:, 0:2],
                             start=(kc == 0), stop=(kc == DC - 1))
        osb = fp.tile([1, 2], F32, tag='osb', name='osb')
        nc.scalar.activation(osb[:], po[:], AF.Identity,
                             bias=chw[0:1, COLS['hb2']:COLS['hb2'] + 1])
        nc.sync.dma_start(out_d[:], osb[:])

    return nc


# ---------------------------------------------------------------- host wrapper
def kernel(**inputs):
    inp = {k: np.asarray(v) for k, v in inputs.items()}
    B = inp['sig_n'].shape[0]
    assert B == 16, f'expected B=16, got {B}'

    sig = inp['sig_n'].astype(np.float32)
    x = np.where(np.isfinite(sig), sig, 0.0)
    x = np.where(x == -1.0, 0.0, x).astype(np.float32)
    valid = np.any(sig != -1.0, axis=1)                # [16, 1000]

    xh = np.zeros((B, 4, L + 8), np.float32)
    xh[:, :, 4:4 + L] = x
    t1full = np.zeros((B, 36, NPAD), np.float32)
    for t in range(9):
        t1full[:, 4 * t:4 * t + 4, 0:L] = xh[:, :, t:t + L]

    meta = inp['meta'].astype(np.int64)
    e_chr = inp['emb_chr'][np.clip(meta[:, 2], 0, 22)]
    e_gene = inp['emb_gene'][np.maximum(inp['gene_id'].astype(np.int64), 0)]
    e_exon = inp['emb_exon'][np.clip(inp['exon_id'].astype(np.int64), 0, 128)]
    e_ctx = (e_chr + e_gene + e_exon).astype(np.float32)

    pos = np.arange(L, dtype=np.float32)[:, None]
    div = np.exp(np.arange(0, D, 2, dtype=np.float32) * (-np.log(10000.0) / D))
    pe = np.zeros((L, D), np.float32)
    pe[:, 0::2] = np.sin(pos * div)
    pe[:, 1::2] = np.cos(pos * div)

    ADD = np.zeros((B, D, NPAD), np.float32)
    ADD[:, :, 0] = inp['cls'][0, 0][None, :] + e_ctx
    ADD[:, :, 1:1 + L] = pe.T[None] + e_ctx[:, :, None]

    w1t = np.zeros((36, 64), np.float32)
    for t in range(9):
        w1t[4 * t:4 * t + 4] = inp['conv1_w'][:, :, t].T
    w2t = np.ascontiguousarray(inp['conv2_w'].transpose(1, 2, 0)).astype(np.float32)
    w3t = np.ascontiguousarray(inp['conv3_w'][:, :, 0].T).astype(np.float32)
    gmt = np.zeros((8, P), np.float32)
    for p in range(P):
        gmt[p // 16, p] = 1.0
    projt = np.ascontiguousarray((inp['proj'] * DN).transpose(0, 2, 1)).astype(np.float32)

    bf = lambda a: np.ascontiguousarray(np.asarray(a, np.float32).astype(ml_dtypes.bfloat16))
    shared = dict(
        w1t=w1t, w2t=w2t, w3t=w3t, gmt=gmt, projt=projt,
        wqb=bf(inp['wq']), wkb=bf(inp['wk']), wvb=bf(inp['wv']),
        wob=bf(inp['wo']),
        f1b=bf(inp['ff_w1']), f2b=bf(inp['ff_w2']),
        hw1b=bf(inp['h_w1']),
        hw2=np.ascontiguousarray(inp['h_w2'], dtype=np.float32),
    )

    in_maps = []
    for c in range(N_CORES):
        b0 = 2 * c
        real01 = np.zeros((P, 16), np.float32)
        vm01 = np.zeros((P, 16), np.float32)
        for s in range(2):
            for n in range(NPAD):
                t, row = n // P, n % P
                if n <= L:
                    real01[row, s * 8 + t] = 1.0
                    if n == 0 or valid[b0 + s, n - 1]:
                        vm01[row, s * 8 + t] = 1.0
        chw = pack_chw(inp, real01, vm01)
        in_maps.append(dict(
            shared,
            xh=np.ascontiguousarray(t1full[b0:b0 + 2]),
            addt=np.ascontiguousarray(ADD[b0:b0 + 2]),
            chw=chw,
        ))

    nc = bacc.Bacc()
    build(nc)
    nc.finalize()
    res = run_bass_kernel_spmd(nc, in_maps, list(range(N_CORES)))
    global LAST_RESULT
    LAST_RESULT = res
    out = np.concatenate([np.asarray(res.results[c]['o']).reshape(2)
                          for c in range(N_CORES)])
    return out.astype(np.float32)


LAST_RESULT = None


if __name__ == '__main__':
    import reference
    inputs = {k: np.asarray(v) for k, v in reference.setup_inputs().items()}
    got = kernel(**inputs)
    print('kernel out:', got)


# revision 6
# speedup vs baseline: 1.1782x; 1.0702x over previous
"""Trainium2 Bass kernel for nn_CNVRegressor (CNN tokenizer + 5-layer Performer + head).

Sharding: data-parallel over batch B=16 across 8 cores (2 samples/core).
Layout: feature-major activations [D on partitions, tokens on free].
Per-sample sequence padded 1001 -> 1024; two samples side by side -> [512, 2048].

Single-pass FAVOR+: kp = exp(ddk - diagk) is computed unstabilized (safe in
f32/bf16 since |ddk| <~ 10); the reference's global key-stab enters ONLY via
the eps-term coefficient gamma = eps*exp(stab_g). The per-core max is taken as
a cheap byproduct of the kp tiles (DVE max + log), AllReduce(max)'d across the
8 cores while the q-side computes, then folded into the rank-1 eps correction.
This removes the old dedicated k-stab pass (a full K projection + 266-wide
scan per layer) with no numerical change.

Self-contained: hardcodes all shapes; host does only input prep / sharding
(cleanup, halo pad, embedding row gather, PE table, bias/mask packing).
"""
import math
from contextlib import ExitStack

import ml_dtypes
import numpy as np

import concourse.bass as bass
import concourse.bacc as bacc
import concourse.tile as tile
from concourse import mybir
from concourse.bass_isa import ReduceOp
from concourse.bass_utils import run_bass_kernel_spmd
from concourse.masks import make_identity

F32 = mybir.dt.float32
F32R = mybir.dt.float32r
BF16 = mybir.dt.bfloat16
AF = mybir.ActivationFunctionType
OP = mybir.AluOpType
AX = mybir.AxisListType

P = 128
D = 512
DH = 64
DEPTH = 5
L = 1000
M = 266
NPAD = 1024
N2 = 2 * NPAD
NT = NPAD // P            # 8 token tiles per sample
DC = D // P               # 4 feature chunks
BLK = 512
DN = DH ** -0.25
DNS = DN * math.sqrt(0.5)
LN_EPS = 1e-5
GN_EPS = 1e-5
LOG_EPS = float(np.log(1e-4))
MCH = ((0, 128), (128, 128), (256, 10))   # m-chunks of 266
N_CORES = 8


def build_cols():
    cols, idx = {}, 0

    def a(name, n):
        nonlocal idx
        cols[name] = idx
        idx += n

    for n in ('gn_w', 'gn_b', 'lnt_w', 'lnt_b'):
        a(n, 4)
    a('b1', 1); a('b2', 1); a('b3', 4); a('gmask', 8)
    a('real', 16); a('stabb', 16); a('vmask', 16)
    for n in ('hln_w', 'hln_b', 'hb1'):
        a(n, 4)
    a('hb2', 1)
    a('cln', 1)
    a('clog', 1)
    a('ctiny', 1)
    for l in range(DEPTH):
        for n in ('ln1w', 'ln1b', 'ln2w', 'ln2b', 'bq', 'bk', 'bv', 'bo', 'fb2'):
            a(f'{n}{l}', 4)
        a(f'fb1{l}', 16)
    return cols, idx


COLS, NCOL = build_cols()


def pack_chw(inp, real01, vmask01):
    chw = np.zeros((P, NCOL), np.float32)

    def put(name, vec):
        vec = np.asarray(vec, np.float32).reshape(-1)
        c0 = COLS[name]
        for c in range((len(vec) + P - 1) // P):
            seg = vec[c * P:(c + 1) * P]
            chw[:len(seg), c0 + c] = seg

    put('gn_w', inp['gn_w']); put('gn_b', inp['gn_b'])
    put('lnt_w', inp['lnt_w']); put('lnt_b', inp['lnt_b'])
    put('b1', inp['conv1_b']); put('b2', inp['conv2_b']); put('b3', inp['conv3_b'])
    gm = np.zeros((P, 8), np.float32)
    for p in range(P):
        gm[p, p // 16] = 1.0
    chw[:, COLS['gmask']:COLS['gmask'] + 8] = gm
    chw[:, COLS['real']:COLS['real'] + 16] = real01
    chw[:, COLS['stabb']:COLS['stabb'] + 16] = (real01 - 1.0) * 1e30
    chw[:, COLS['vmask']:COLS['vmask'] + 16] = vmask01
    put('hln_w', inp['h_ln_w']); put('hln_b', inp['h_ln_b'])
    put('hb1', inp['h_b1']); put('hb2', inp['h_b2'])
    chw[:, COLS['cln']] = LN_EPS
    chw[:, COLS['clog']] = LOG_EPS
    chw[:, COLS['ctiny']] = 1e-30
    for l in range(DEPTH):
        put(f'ln1w{l}', inp['ln1_w'][l]); put(f'ln1b{l}', inp['ln1_b'][l])
        put(f'ln2w{l}', inp['ln2_w'][l]); put(f'ln2b{l}', inp['ln2_b'][l])
        put(f'bq{l}', inp['bq'][l]); put(f'bk{l}', inp['bk'][l])
        put(f'bv{l}', inp['bv'][l]); put(f'bo{l}', inp['bo'][l])
        put(f'fb1{l}', inp['ff_b1'][l]); put(f'fb2{l}', inp['ff_b2'][l])
    return chw


def blocks(width, bs=BLK, off0=0):
    out, off = [], 0
    while off < width:
        out.append((off0 + off, min(bs, width - off)))
        off += bs
    return out


# ---------------------------------------------------------------- device build
def build(nc):
    dp = lambda n, sh, dt=F32: nc.declare_dram_parameter(n, sh, dt, isOutput=False)
    xh_d = dp('xh', (2, 36, NPAD))
    add_d = dp('addt', (2, D, NPAD))
    w1t_d = dp('w1t', (36, 64))
    w2t_d = dp('w2t', (64, 9, 128))
    w3t_d = dp('w3t', (128, D))
    chw_d = dp('chw', (P, NCOL))
    gmt_d = dp('gmt', (8, P))
    proj_d = dp('projt', (DEPTH, DH, M))
    wq_d = dp('wqb', (DEPTH, D, D), BF16)
    wk_d = dp('wkb', (DEPTH, D, D), BF16)
    wv_d = dp('wvb', (DEPTH, D, D), BF16)
    wob_d = dp('wob', (DEPTH, D, D), BF16)
    f1_d = dp('f1b', (DEPTH, D, 4 * D), BF16)
    f2_d = dp('f2b', (DEPTH, 4 * D, D), BF16)
    hw1_d = dp('hw1b', (D, D), BF16)
    hw2_d = dp('hw2', (D, 1))
    out_d = nc.declare_dram_parameter('o', (1, 2), F32, isOutput=True)

    with tile.TileContext(nc) as tc, ExitStack() as ctx:
        const = ctx.enter_context(tc.tile_pool(name='const', bufs=1))
        sp = ctx.enter_context(tc.tile_pool(name='sp', bufs=1))
        actp = ctx.enter_context(tc.tile_pool(name='actp', bufs=1))
        wpp = ctx.enter_context(tc.tile_pool(name='wpp', bufs=2))
        fp = ctx.enter_context(tc.tile_pool(name='fp', bufs=2))
        kvp = ctx.enter_context(tc.tile_pool(name='kvp', bufs=3))
        dram = ctx.enter_context(tc.tile_pool(name='dram', bufs=2, space='DRAM'))
        pmm = ctx.enter_context(tc.tile_pool(name='pmm', bufs=4, space='PSUM'))
        ps1 = ctx.enter_context(tc.tile_pool(name='ps1', bufs=3, space='PSUM'))

        # ---- constants
        chw = const.tile([P, NCOL], F32, name='chw')
        nc.sync.dma_start(chw[:], chw_d[:])
        cw = lambda name, off=0: chw[:, COLS[name] + off:COLS[name] + off + 1]
        cwp = lambda name, parts: chw[0:parts, COLS[name]:COLS[name] + 1]
        ident = const.tile([P, P], F32, name='ident')
        make_identity(nc, ident)
        identb = const.tile([P, P], BF16, name='identb')
        nc.vector.tensor_copy(identb[:], ident[:])
        ones = const.tile([P, 1], F32, name='ones')
        nc.vector.memset(ones[:], 1.0)
        onesb = const.tile([P, 1], BF16, name='onesb')
        nc.vector.memset(onesb[:], 1.0)
        gmt = const.tile([8, P], F32, name='gmt')
        nc.sync.dma_start(gmt[:], gmt_d[:])
        w1t = const.tile([36, 64], F32, name='w1t')
        nc.sync.dma_start(w1t[:], w1t_d[:])
        w2t = const.tile([64, 9, 128], F32, name='w2t')
        nc.sync.dma_start(w2t[:], w2t_d[:])
        w3t = const.tile([128, D], F32, name='w3t')
        nc.sync.dma_start(w3t[:], w3t_d[:])

        S = [sp.tile([P, N2], F32, tag=f'S{c}', name=f'S{c}') for c in range(DC)]

        # -------------------------------------------------------- shared LN
        def ln_fm(X, wc, bc, col0, width, ytag):
            """Per-token LN over the 512 partition dim (feature-major).

            X tiles are f32; returns bf16 normed tiles."""
            Y = [actp.tile([P, N2], BF16, tag=f'{ytag}{c}', name=f'{ytag}{c}')
                 for c in range(DC)]
            for c in range(DC):
                nc.vector.tensor_mul(Y[c][:, col0:col0 + width],
                                     X[c][:, col0:col0 + width],
                                     X[c][:, col0:col0 + width])
            srow = fp.tile([1, N2], F32, tag='srow', bufs=1, name='srow')
            qrow = fp.tile([1, N2], F32, tag='qrow', bufs=1, name='qrow')
            trow = fp.tile([1, N2], F32, tag='trow', bufs=1, name='trow')
            mrow, vrow = srow, qrow
            for o, bw in blocks(width, BLK, col0):
                ps = ps1.tile([1, BLK], F32, tag='ps1', name='ps')
                pq = ps1.tile([1, BLK], F32, tag='ps1', name='pq')
                for c in range(DC):
                    nc.tensor.matmul(ps[:, :bw], ones[:], X[c][:, o:o + bw],
                                     start=(c == 0), stop=(c == DC - 1))
                    nc.tensor.matmul(pq[:, :bw], onesb[:], Y[c][:, o:o + bw],
                                     start=(c == 0), stop=(c == DC - 1))
                nc.any.tensor_copy(srow[:, o:o + bw], ps[:, :bw])
                nc.any.tensor_copy(qrow[:, o:o + bw], pq[:, :bw])
            sl = slice(col0, col0 + width)
            nc.vector.tensor_scalar_mul(mrow[:, sl], srow[:, sl], 1.0 / D)
            nc.vector.tensor_mul(trow[:, sl], mrow[:, sl], mrow[:, sl])
            nc.vector.tensor_scalar(vrow[:, sl], qrow[:, sl], 1.0 / D, None,
                                    OP.mult)
            nc.vector.tensor_sub(vrow[:, sl], vrow[:, sl], trow[:, sl])
            nc.scalar.activation(vrow[:, sl], vrow[:, sl], AF.Ln,
                                 bias=cwp('cln', 1))
            nc.scalar.activation(vrow[:, sl], vrow[:, sl], AF.Exp, scale=-0.5)
            MU = actp.tile([P, N2], F32, tag='MU', name='MU')
            RS = actp.tile([P, N2], F32, tag='RS', name='RS')
            nc.gpsimd.partition_broadcast(MU[:, sl], mrow[:, sl], P)
            nc.gpsimd.partition_broadcast(RS[:, sl], vrow[:, sl], P)
            for c in range(DC):
                nc.vector.tensor_sub(Y[c][:, sl], X[c][:, sl], MU[:, sl])
                nc.vector.tensor_mul(Y[c][:, sl], Y[c][:, sl], RS[:, sl])
                nc.scalar.activation(Y[c][:, sl], Y[c][:, sl], AF.Identity,
                                     scale=cw(wc, c), bias=cw(bc, c))
            return Y

        # full-width projection: dst[128, N2] = (w^T y) + bias, both heads of
        # a pair. kc-outer so each stationary is loaded once per 4 blocks.
        def proj_mm(wd, l, hp, bn, dst):
            wt = wpp.tile([P, DC, P], BF16, tag='wpq', name='wt')
            nc.sync.dma_start(
                wt[:], wd[l, :, hp * P:(hp + 1) * P]
                .rearrange('(kc p) m -> p kc m', p=P))
            bias = chw[:, COLS[f'{bn}{l}'] + hp:COLS[f'{bn}{l}'] + hp + 1]
            pms = [pmm.tile([P, BLK], F32, tag='pmm', name=f'pm{bi}')
                   for bi in range(4)]
            for kc in range(DC):
                for bi, (b, bw) in enumerate(blocks(N2)):
                    nc.tensor.matmul(pms[bi][:], wt[:, kc, :], y[kc][:, b:b + bw],
                                     start=(kc == 0), stop=(kc == DC - 1))
            for bi, (b, bw) in enumerate(blocks(N2)):
                nc.scalar.activation(dst[:, b:b + bw], pms[bi][:], AF.Identity,
                                     bias=bias)

        # -------------------------------------------------------- tokenizer
        for s in range(2):
            for c in range(DC):
                nc.sync.dma_start(S[c][:, s * NPAD:(s + 1) * NPAD],
                                  add_d[s, c * P:(c + 1) * P, :])
        for s in range(2):
            t1 = fp.tile([36, NPAD], F32, tag='tokA', bufs=1, name='t1')
            nc.sync.dma_start(t1[:], xh_d[s])
            y1h = fp.tile([64, L + 8], F32, tag='tokB', bufs=1, name='y1h')
            nc.vector.memset(y1h[:], 0.0)
            for o, bw in blocks(L):
                p1 = pmm.tile([64, BLK], F32, tag='pmm', name='p1')
                nc.tensor.matmul(p1[:, :bw], w1t[:], t1[:, o:o + bw],
                                 start=True, stop=True)
                nc.scalar.activation(y1h[:, 4 + o:4 + o + bw], p1[:, :bw], AF.Gelu,
                                     bias=chw[0:64, COLS['b1']:COLS['b1'] + 1])
            y2 = fp.tile([P, NPAD], F32, tag='tokA', bufs=1, name='y2')
            for o, bw in blocks(L):
                p2 = pmm.tile([P, BLK], F32, tag='pmm', name='p2')
                for t in range(9):
                    nc.tensor.matmul(p2[:, :bw], w2t[:, t, :],
                                     y1h[:, t + o:t + o + bw],
                                     start=(t == 0), stop=(t == 8))
                nc.scalar.activation(y2[:, o:o + bw], p2[:, :bw], AF.Gelu,
                                     bias=cw('b2'))
            x3 = [actp.tile([P, NPAD], F32, tag=f'A{c}', name=f'x3{c}')
                  for c in range(DC)]
            for c in range(DC):
                for o, bw in blocks(L):
                    p3 = pmm.tile([P, BLK], F32, tag='pmm', name='p3')
                    nc.tensor.matmul(p3[:, :bw], w3t[:, c * P:(c + 1) * P],
                                     y2[:, o:o + bw], start=True, stop=True)
                    nc.scalar.activation(x3[c][:, o:o + bw], p3[:, :bw],
                                         AF.Identity, bias=cw('b3', c))
            # GroupNorm(32, 512) over [16ch x 1000]
            stats = fp.tile([P, 8], F32, tag='gstats', name='stats')
            sqt = fp.tile([P, NPAD], F32, tag='tokB', bufs=1, name='sqt')
            for c in range(DC):
                nc.vector.tensor_reduce(stats[:, c:c + 1], x3[c][:, 0:L], AX.X, OP.add)
                nc.vector.tensor_mul(sqt[:, 0:L], x3[c][:, 0:L], x3[c][:, 0:L])
                nc.vector.tensor_reduce(stats[:, 4 + c:5 + c], sqt[:, 0:L], AX.X, OP.add)
            pg = ps1.tile([8, 8], F32, tag='ps1', name='pg')
            nc.tensor.matmul(pg[:], chw[:, COLS['gmask']:COLS['gmask'] + 8],
                             stats[:], start=True, stop=True)
            gs = fp.tile([8, 8], F32, tag='gs', name='gs')
            nc.vector.tensor_scalar_mul(gs[:], pg[:], 1.0 / (16 * L))
            gm2 = fp.tile([8, 4], F32, tag='gm2', name='gm2')
            nc.vector.tensor_mul(gm2[:], gs[:, 0:4], gs[:, 0:4])
            nc.vector.tensor_sub(gs[:, 4:8], gs[:, 4:8], gm2[:])
            nc.scalar.activation(gs[:, 4:8], gs[:, 4:8], AF.Ln, bias=cwp('cln', 8))
            nc.scalar.activation(gs[:, 4:8], gs[:, 4:8], AF.Exp, scale=-0.5)
            pb = ps1.tile([P, 8], F32, tag='ps1', name='pb')
            nc.tensor.matmul(pb[:], gmt[:], gs[:], start=True, stop=True)
            cstat = fp.tile([P, 8], F32, tag='cstat', name='cstat')
            nc.vector.tensor_copy(cstat[:], pb[:])
            for c in range(DC):
                nc.vector.tensor_scalar(x3[c][:, 0:L], x3[c][:, 0:L],
                                        cstat[:, c:c + 1], cstat[:, 4 + c:5 + c],
                                        OP.subtract, OP.mult)
                nc.scalar.activation(x3[c][:, 0:L], x3[c][:, 0:L], AF.Identity,
                                     scale=cw('gn_w', c), bias=cw('gn_b', c))
            tok = ln_fm(x3, 'lnt_w', 'lnt_b', 0, L, 'y')
            b0 = s * NPAD
            for c in range(DC):
                nc.vector.tensor_add(S[c][:, b0 + 1:b0 + 1 + L],
                                     S[c][:, b0 + 1:b0 + 1 + L], tok[c][:, 0:L])

        # -------------------------------------------------------- layers
        for l in range(DEPTH):
            projT = fp.tile([DH, M], F32, tag='projT', name='projT')
            nc.sync.dma_start(projT[:], proj_d[l])
            # duplicated into both partition halves so head-1 slices
            # (base partition 64) can pair with it in matmuls; the second
            # half is filled by DMA (engines can't shift partitions)
            projTb = fp.tile([P, M], BF16, tag='projTb', name='projTb')
            nc.vector.tensor_copy(projTb[0:DH, :], projT[:])
            nc.sync.dma_start(projTb[DH:P, :], projTb[0:DH, :])

            y = ln_fm(S, f'ln1w{l}', f'ln1b{l}', 0, N2, 'y')

            # per-layer k-side context accumulators [65, 272] x 16 head-samples
            ctxE = fp.tile([65, 16, 272], BF16, tag='ctxE', bufs=1, name='ctxE')
            smax = fp.tile([P, 16], F32, tag='smax', name='smax')

            # ---- phase K: k/v projections, kp, ctx/ksum, local stab byproduct
            for hp in range(4):
                kh2 = fp.tile([P, N2], BF16, tag='kh2', bufs=2, name='kh2')
                vh2 = fp.tile([P, N2], BF16, tag='vh2', bufs=2, name='vh2')
                proj_mm(wk_d, l, hp, 'bk', kh2)
                proj_mm(wv_d, l, hp, 'bv', vh2)
                for hh in range(2):
                    hsl = slice(hh * DH, (hh + 1) * DH)
                    for s in range(2):
                        base = s * NPAD
                        idx = hp * 4 + hh * 2 + s
                        # --- -diag_k per token, [P, 8]; pad rows get -1e30
                        biask = fp.tile([P, 8], F32, tag='biask', name='biask')
                        for t in range(NT):
                            csl = slice(base + t * P, base + (t + 1) * P)
                            pkt = ps1.tile([P, 64], BF16, tag='ps1', name='pkt')
                            nc.tensor.transpose(pkt[:], kh2[hsl, csl],
                                                identb[hsl, hsl])
                            sqk = fp.tile([P, 64], F32, tag='sqk', name='sqk')
                            nc.scalar.activation(sqk[:], pkt[:], AF.Square,
                                                 scale=DNS)
                            nc.vector.tensor_reduce(biask[:, t:t + 1], sqk[:],
                                                    AX.X, OP.add)
                        nc.vector.tensor_scalar_mul(biask[:], biask[:], -1.0)
                        bmask = fp.tile([P, 8], F32, tag='bmask', name='bmask')
                        nc.vector.tensor_add(
                            bmask[:], biask[:],
                            chw[:, COLS['stabb'] + s * 8:COLS['stabb'] + s * 8 + 8])
                        # --- kp tiles + ctx'^T [65, 267] accumulation
                        pctx = ps1.tile([65, 272], F32, tag='psx', bufs=1,
                                        name='pctx')
                        rm8 = fp.tile([P, 8], F32, tag='rm8', name='rm8')
                        for t in range(NT):
                            csl = slice(base + t * P, base + (t + 1) * P)
                            pdk = ps1.tile([P, 272], F32, tag='ps1', name='pdk')
                            nc.tensor.matmul(pdk[:, 0:M], kh2[hsl, csl],
                                             projTb[hsl, :], start=True, stop=True)
                            kp = kvp.tile([P, 272], BF16, tag='kp', name='kp')
                            nc.scalar.activation(kp[:, 0:M], pdk[:, 0:M], AF.Exp,
                                                 bias=bmask[:, t:t + 1])
                            nc.vector.tensor_copy(kp[:, M:M + 1],
                                                  cw('real', s * 8 + t))
                            nc.vector.tensor_reduce(rm8[:, t:t + 1], kp[:, 0:M],
                                                    AX.X, OP.max)
                            pvt = ps1.tile([P, 64], BF16, tag='ps1', name='pvt')
                            nc.tensor.transpose(pvt[:], vh2[hsl, csl],
                                                identb[hsl, hsl])
                            v1 = kvp.tile([P, 65], BF16, tag='v1', name='v1')
                            nc.vector.tensor_scalar_mul(v1[:, 0:64], pvt[:],
                                                        cw('vmask', s * 8 + t))
                            nc.vector.memset(v1[:, 64:65], 1.0)
                            nc.tensor.matmul(pctx[:, 0:M + 1], v1[:], kp[:, 0:M + 1],
                                             start=(t == 0), stop=(t == NT - 1))
                        nc.vector.tensor_copy(ctxE[:, idx, 0:M + 1],
                                              pctx[:, 0:M + 1])
                        # --- local stab byproduct, kept in the exp domain:
                        # max_m e^{dd} = (max_m kp) * e^{diag}. Exp shares the
                        # kp table (no ACT table thrash); pad rows stay 0.
                        m8 = fp.tile([P, 8], F32, tag='m8', name='m8')
                        nc.scalar.activation(m8[:], biask[:], AF.Exp,
                                             scale=-1.0)
                        nc.vector.tensor_mul(m8[:], m8[:], rm8[:])
                        nc.vector.tensor_reduce(smax[:, idx:idx + 1], m8[:],
                                                AX.X, OP.max)

            # ---- global key-stab: AllReduce(max) across the 8 cores.
            # Overlaps with the q-side below (consumed only at ctxT/vsc).
            sfin = fp.tile([P, 1], F32, tag='sfin', name='sfin')
            nc.vector.tensor_reduce(sfin[:], smax[:], AX.X, OP.max)
            nc.gpsimd.partition_all_reduce(sfin[:], sfin[:], P, ReduceOp.max)
            bin_ = dram.tile([P, 1], F32, name='bin')
            bout = dram.tile([P, 1], F32, name='bout')
            nc.sync.dma_start(bin_[:], sfin[:])
            nc.gpsimd.collective_compute(
                'AllReduce', OP.max,
                replica_groups=[list(range(N_CORES))],
                ins=[bin_.opt()], outs=[bout.opt()])
            stabg = fp.tile([P, 1], F32, tag='stabg', name='stabg')
            nc.sync.dma_start(stabg[:], bout[:])
            # stabg already holds e^{stab_g}; gamma = eps * e^{stab_g}
            ceps65 = fp.tile([65, 1], F32, tag='ceps65', name='ceps65')
            nc.vector.tensor_scalar_mul(ceps65[:], stabg[0:65, :], 1e-4)

            # ---- phase Q: q projection, qp, eps corrections, num/den -> A
            A = [actp.tile([P, N2], BF16, tag=f'A{c}', name=f'Aa{c}')
                 for c in range(DC)]
            for hp in range(4):
                qh2 = fp.tile([P, N2], BF16, tag='qh2', bufs=2, name='qh2')
                proj_mm(wq_d, l, hp, 'bq', qh2)
                # (dn^2/2)*q^2 for diag_q, both heads/samples in one op
                sqq = fp.tile([P, N2], BF16, tag='sqq', bufs=1, name='sqq')
                nc.scalar.activation(sqq[:], qh2[:], AF.Square, scale=DNS)
                for hh in range(2):
                    hsl = slice(hh * DH, (hh + 1) * DH)
                    for s in range(2):
                        base = s * NPAD
                        idx = hp * 4 + hh * 2 + s
                        # --- qp = exp(ddq), feature-major
                        qp = [fp.tile([P, NPAD], BF16, tag='qp0', bufs=1, name='qp0'),
                              fp.tile([P, NPAD], BF16, tag='qp1', bufs=1, name='qp1'),
                              fp.tile([10, NPAD], BF16, tag='qp2', bufs=1, name='qp2')]
                        for ci, (m0, mw) in enumerate(MCH):
                            for b, bw in blocks(NPAD):
                                pdq = pmm.tile([P, BLK], F32, tag='pmm', name='pdq')
                                nc.tensor.matmul(
                                    pdq[0:mw, :], projTb[hsl, m0:m0 + mw],
                                    qh2[hsl, base + b:base + b + bw],
                                    start=True, stop=True)
                                nc.scalar.activation(qp[ci][0:mw, b:b + bw],
                                                     pdq[0:mw, :], AF.Exp)
                        # --- colmax = e^{stab_q} per token: fold the 3 chunks
                        # with same-partition DVE maxes, then ONE gpsimd
                        # all-reduce (was three)
                        ar = fp.tile([P, NPAD], BF16, tag='ar', bufs=1, name='ar')
                        nc.vector.tensor_tensor(ar[:], qp[0][:], qp[1][:], OP.max)
                        nc.vector.tensor_tensor(ar[0:10, :], ar[0:10, :],
                                                qp[2][:], OP.max)
                        nc.gpsimd.partition_all_reduce(ar[:], ar[:], P,
                                                       ReduceOp.max)
                        # --- epsE = eps * e^{diag_q} * colmax
                        epsE = fp.tile([1, NPAD], BF16, tag='epsE', bufs=1,
                                       name='epsE')
                        for b, bw in blocks(NPAD):
                            pq2 = ps1.tile([1, BLK], F32, tag='ps1', name='pq2')
                            nc.tensor.matmul(pq2[:, :bw], onesb[hsl, :],
                                             sqq[hsl, base + b:base + b + bw],
                                             start=True, stop=True)
                            nc.scalar.activation(epsE[:, b:b + bw], pq2[:, :bw],
                                                 AF.Exp, bias=cwp('clog', 1))
                        nc.vector.tensor_mul(epsE[:], epsE[:], ar[0:1, :])
                        # --- ctxT = ctxE + gamma * vsum; -> [266(+1), 65] chunks
                        vsc = fp.tile([65, 1], F32, tag='vsc', name='vsc')
                        nc.vector.tensor_mul(vsc[:], ctxE[:, idx, M:M + 1],
                                             ceps65[:])
                        ctxT = fp.tile([65, M], BF16, tag='ctxT', name='ctxT')
                        nc.vector.tensor_scalar(ctxT[:], ctxE[:, idx, 0:M],
                                                vsc[:], None, OP.add)
                        ctx_sb = []
                        for ci, (m0, mw) in enumerate(MCH):
                            ptc = ps1.tile([P, 65], BF16, tag='ps1', name='ptc')
                            nc.tensor.transpose(ptc[0:mw, :], ctxT[:, m0:m0 + mw],
                                                identb[0:65, 0:65])
                            csb = fp.tile([P, 65], BF16, tag=f'ctx{ci}', name=f'c{ci}')
                            nc.any.tensor_copy(csb[0:mw, :], ptc[0:mw, :])
                            ctx_sb.append(csb)
                        # csr = colsum of ctxT, for the rank-1 eps term
                        pcs = ps1.tile([1, 65], F32, tag='ps1', name='pcs')
                        for ci, (m0, mw) in enumerate(MCH):
                            nc.tensor.matmul(pcs[:], onesb[0:mw, :],
                                             ctx_sb[ci][0:mw, :],
                                             start=(ci == 0), stop=(ci == 2))
                        csr = fp.tile([1, 65], BF16, tag='csr', name='csr')
                        nc.any.tensor_copy(csr[:], pcs[:])
                        # --- num_den [65, n]; rows 0..63 num, row 64 den
                        for b, bw in blocks(NPAD):
                            pnd = ps1.tile([65, BLK], F32, tag='ps1', name='pnd')
                            for ci, (m0, mw) in enumerate(MCH):
                                nc.tensor.matmul(pnd[:], ctx_sb[ci][0:mw, :],
                                                 qp[ci][0:mw, b:b + bw],
                                                 start=(ci == 0), stop=False)
                            nc.tensor.matmul(pnd[:], csr[:], epsE[:, b:b + bw],
                                             start=False, stop=True)
                            dinv = fp.tile([1, BLK], F32, tag='dinv', bufs=1,
                                           name='dinv')
                            nc.vector.reciprocal(dinv[:], pnd[64:65, :])
                            dvb = fp.tile([64, BLK], F32, tag='dvb', bufs=1,
                                          name='dvb')
                            nc.gpsimd.partition_broadcast(dvb[:], dinv[:], 64)
                            nc.vector.tensor_mul(
                                A[hp][hsl, base + b:base + b + bw],
                                pnd[0:64, :], dvb[:])

            # ---- wo: S += A @ wo + bo (kc-outer for stationary reuse)
            for mc in range(DC):
                wt = wpp.tile([P, DC, P], BF16, tag='wpo', name='wto')
                nc.sync.dma_start(
                    wt[:], wob_d[l, :, mc * P:(mc + 1) * P]
                    .rearrange('(kc p) m -> p kc m', p=P))
                pms = [pmm.tile([P, BLK], F32, tag='pmm', name=f'pmo{bi}')
                       for bi in range(4)]
                for kc in range(DC):
                    for bi, (b, bw) in enumerate(blocks(N2)):
                        nc.tensor.matmul(pms[bi][:], wt[:, kc, :],
                                         A[kc][:, b:b + bw],
                                         start=(kc == 0), stop=(kc == DC - 1))
                for bi, (b, bw) in enumerate(blocks(N2)):
                    nc.vector.tensor_add(S[mc][:, b:b + bw], S[mc][:, b:b + bw],
                                         pms[bi][:])
                nc.scalar.activation(S[mc][:], S[mc][:], AF.Identity,
                                     bias=cw(f'bo{l}', mc))

            # ---- FF in quarters of the 2048 hidden dim (kc-outer)
            y2t = ln_fm(S, f'ln2w{l}', f'ln2b{l}', 0, N2, 'y')
            for q in range(4):
                w1q = fp.tile([P, DC, BLK], BF16, tag='w1q', bufs=1, name='w1q')
                nc.sync.dma_start(
                    w1q[:], f1_d[l, :, q * BLK:(q + 1) * BLK]
                    .rearrange('(kc p) m -> p kc m', p=P))
                w2q = fp.tile([P, DC, BLK], BF16, tag='w2q', bufs=1, name='w2q')
                nc.sync.dma_start(
                    w2q[:], f2_d[l, q * BLK:(q + 1) * BLK, :]
                    .rearrange('(kc p) m -> p kc m', p=P))
                # reuse the dead A tiles (same shape) for the FF hidden
                H = [actp.tile([P, N2], BF16, tag=f'A{mc}', name=f'H{mc}')
                     for mc in range(DC)]
                for mc in range(DC):
                    pms = [pmm.tile([P, BLK], F32, tag='pmm', name=f'pmf1{bi}')
                           for bi in range(4)]
                    for kc in range(DC):
                        for bi, (b, bw) in enumerate(blocks(N2)):
                            nc.tensor.matmul(pms[bi][:],
                                             w1q[:, kc, mc * P:(mc + 1) * P],
                                             y2t[kc][:, b:b + bw],
                                             start=(kc == 0), stop=(kc == DC - 1))
                    for bi, (b, bw) in enumerate(blocks(N2)):
                        nc.scalar.activation(H[mc][:, b:b + bw], pms[bi][:],
                                             AF.Gelu, bias=cw(f'fb1{l}', q * 4 + mc))
                for mc in range(DC):
                    pms = [pmm.tile([P, BLK], F32, tag='pmm', name=f'pmf2{bi}')
                           for bi in range(4)]
                    for kc in range(DC):
                        for bi, (b, bw) in enumerate(blocks(N2)):
                            nc.tensor.matmul(pms[bi][:],
                                             w2q[:, kc, mc * P:(mc + 1) * P],
                                             H[kc][:, b:b + bw],
                                             start=(kc == 0), stop=(kc == DC - 1))
                    for bi, (b, bw) in enumerate(blocks(N2)):
                        nc.vector.tensor_add(S[mc][:, b:b + bw],
                                             S[mc][:, b:b + bw], pms[bi][:])
            for mc in range(DC):
                nc.scalar.activation(S[mc][:], S[mc][:], AF.Identity,
                                     bias=cw(f'fb2{l}', mc))
                for s in range(2):
                    nc.vector.memset(S[mc][:, s * NPAD + 1 + L:(s + 1) * NPAD], 0.0)

        # -------------------------------------------------------- head
        clsx = [fp.tile([P, 2], F32, tag=f'cls{c}', name=f'cls{c}')
                for c in range(DC)]
        for c in range(DC):
            nc.vector.tensor_copy(clsx[c][:, 0:1], S[c][:, 0:1])
            nc.vector.tensor_copy(clsx[c][:, 1:2], S[c][:, NPAD:NPAD + 1])
        hx = ln_fm(clsx, 'hln_w', 'hln_b', 0, 2, 'y')
        hh_t = []
        for mc in range(DC):
            wt = wpp.tile([P, DC, P], BF16, tag='wpq', name='wth')
            nc.sync.dma_start(wt[:], hw1_d[:, mc * P:(mc + 1) * P]
                              .rearrange('(kc p) m -> p kc m', p=P))
            pm = ps1.tile([P, 2], F32, tag='ps1', name='pmh')
            for kc in range(DC):
                nc.tensor.matmul(pm[:], wt[:, kc, :], hx[kc][:, 0:2],
                                 start=(kc == 0), stop=(kc == DC - 1))
            ht = fp.tile([P, 2], F32, tag=f'hh{mc}', name=f'hhd{mc}')
            nc.scalar.activation(ht[:], pm[:], AF.Gelu, bias=cw('hb1', mc))
            hh_t.append(ht)
        wt2 = fp.tile([P, DC, 1], F32, tag='wt2', name='wt2')
        nc.sync.dma_start(wt2[:], hw2_d[:, :].rearrange('(kc p) m -> p kc m', p=P))
        po = ps1.tile([1, 2], F32, tag='ps1', name='po')
        for kc in range(DC):
            nc.tensor.matmul(po[:], wt2[:, kc, :], hh_t[kc][:, 0:2],
                             start=(kc == 0), stop=(kc == DC - 1))
        osb = fp.tile([1, 2], F32, tag='osb', name='osb')
        nc.scalar.activation(osb[:], po[:], AF.Identity,
                             bias=chw[0:1, COLS['hb2']:COLS['hb2'] + 1])
        nc.sync.dma_start(out_d[:], osb[:])

    return nc


# ---------------------------------------------------------------- host wrapper
def kernel(**inputs):
    inp = {k: np.asarray(v) for k, v in inputs.items()}
    B = inp['sig_n'].shape[0]
    assert B == 16, f'expected B=16, got {B}'

    sig = inp['sig_n'].astype(np.float32)
    x = np.where(np.isfinite(sig), sig, 0.0)
    x = np.where(x == -1.0, 0.0, x).astype(np.float32)
    valid = np.any(sig != -1.0, axis=1)                # [16, 1000]

    xh = np.zeros((B, 4, L + 8), np.float32)
    xh[:, :, 4:4 + L] = x
    t1full = np.zeros((B, 36, NPAD), np.float32)
    for t in range(9):
        t1full[:, 4 * t:4 * t + 4, 0:L] = xh[:, :, t:t + L]

    meta = inp['meta'].astype(np.int64)
    e_chr = inp['emb_chr'][np.clip(meta[:, 2], 0, 22)]
    e_gene = inp['emb_gene'][np.maximum(inp['gene_id'].astype(np.int64), 0)]
    e_exon = inp['emb_exon'][np.clip(inp['exon_id'].astype(np.int64), 0, 128)]
    e_ctx = (e_chr + e_gene + e_exon).astype(np.float32)

    pos = np.arange(L, dtype=np.float32)[:, None]
    div = np.exp(np.arange(0, D, 2, dtype=np.float32) * (-np.log(10000.0) / D))
    pe = np.zeros((L, D), np.float32)
    pe[:, 0::2] = np.sin(pos * div)
    pe[:, 1::2] = np.cos(pos * div)

    ADD = np.zeros((B, D, NPAD), np.float32)
    ADD[:, :, 0] = inp['cls'][0, 0][None, :] + e_ctx
    ADD[:, :, 1:1 + L] = pe.T[None] + e_ctx[:, :, None]

    w1t = np.zeros((36, 64), np.float32)
    for t in range(9):
        w1t[4 * t:4 * t + 4] = inp['conv1_w'][:, :, t].T
    w2t = np.ascontiguousarray(inp['conv2_w'].transpose(1, 2, 0)).astype(np.float32)
    w3t = np.ascontiguousarray(inp['conv3_w'][:, :, 0].T).astype(np.float32)
    gmt = np.zeros((8, P), np.float32)
    for p in range(P):
        gmt[p // 16, p] = 1.0
    projt = np.ascontiguousarray((inp['proj'] * DN).transpose(0, 2, 1)).astype(np.float32)

    bf = lambda a: np.ascontiguousarray(np.asarray(a, np.float32).astype(ml_dtypes.bfloat16))
    shared = dict(
        w1t=w1t, w2t=w2t, w3t=w3t, gmt=gmt, projt=projt,
        wqb=bf(inp['wq']), wkb=bf(inp['wk']), wvb=bf(inp['wv']),
        wob=bf(inp['wo']),
        f1b=bf(inp['ff_w1']), f2b=bf(inp['ff_w2']),
        hw1b=bf(inp['h_w1']),
        hw2=np.ascontiguousarray(inp['h_w2'], dtype=np.float32),
    )

    in_maps = []
    for c in range(N_CORES):
        b0 = 2 * c
        real01 = np.zeros((P, 16), np.float32)
        vm01 = np.zeros((P, 16), np.float32)
        for s in range(2):
            for n in range(NPAD):
                t, row = n // P, n % P
                if n <= L:
                    real01[row, s * 8 + t] = 1.0
                    if n == 0 or valid[b0 + s, n - 1]:
                        vm01[row, s * 8 + t] = 1.0
        chw = pack_chw(inp, real01, vm01)
        in_maps.append(dict(
            shared,
            xh=np.ascontiguousarray(t1full[b0:b0 + 2]),
            addt=np.ascontiguousarray(ADD[b0:b0 + 2]),
            chw=chw,
        ))

    nc = bacc.Bacc()
    build(nc)
    nc.finalize()
    res = run_bass_kernel_spmd(nc, in_maps, list(range(N_CORES)))
    global LAST_RESULT
    LAST_RESULT = res
    out = np.concatenate([np.asarray(res.results[c]['o']).reshape(2)
                          for c in range(N_CORES)])
    return out.astype(np.float32)


LAST_RESULT = None


if __name__ == '__main__':
    import reference
    inputs = {k: np.asarray(v) for k, v in reference.setup_inputs().items()}
    got = kernel(**inputs)
    print('kernel out:', got)


# revision 9
# speedup vs baseline: 1.4025x; 1.1905x over previous
"""Trainium2 Bass kernel for nn_CNVRegressor (CNN tokenizer + 5-layer Performer + head).

Sharding: data-parallel over batch B=16 across 8 cores (2 samples/core).
Layout: feature-major activations [D on partitions, tokens on free].
Per-sample sequence padded 1001 -> 1024; two samples side by side -> [512, 2048].

Single-pass FAVOR+: kp = exp(ddk - diagk) is computed unstabilized (safe in
f32/bf16 since |ddk| <~ 10); the reference's global key-stab enters ONLY via
the eps-term coefficient gamma = eps*exp(stab_g). The per-core max is taken as
a cheap byproduct of the kp tiles (DVE max + log), AllReduce(max)'d across the
8 cores while the q-side computes, then folded into the rank-1 eps correction.
This removes the old dedicated k-stab pass (a full K projection + 266-wide
scan per layer) with no numerical change.

Self-contained: hardcodes all shapes; host does only input prep / sharding
(cleanup, halo pad, embedding row gather, PE table, bias/mask packing).
"""
import math
from contextlib import ExitStack

import ml_dtypes
import numpy as np

import concourse.bass as bass
import concourse.bacc as bacc
import concourse.tile as tile
from concourse import mybir
from concourse.bass_isa import ReduceOp
from concourse.bass_utils import run_bass_kernel_spmd
from concourse.masks import make_identity

F32 = mybir.dt.float32
F32R = mybir.dt.float32r
BF16 = mybir.dt.bfloat16
AF = mybir.ActivationFunctionType
OP = mybir.AluOpType
AX = mybir.AxisListType

P = 128
D = 512
DH = 64
DEPTH = 5
L = 1000
M = 266
NPAD = 1024
N2 = 2 * NPAD
NT = NPAD // P            # 8 token tiles per sample
DC = D // P               # 4 feature chunks
BLK = 512
DN = DH ** -0.25
DNS = DN * math.sqrt(0.5)
LN_EPS = 1e-5
GN_EPS = 1e-5
LOG_EPS = float(np.log(1e-4))
MCH = ((0, 128), (128, 128), (256, 10))   # m-chunks of 266
N_CORES = 8


def build_cols():
    cols, idx = {}, 0

    def a(name, n):
        nonlocal idx
        cols[name] = idx
        idx += n

    for n in ('gn_w', 'gn_b', 'lnt_w', 'lnt_b'):
        a(n, 4)
    a('b1', 1); a('b2', 1); a('b3', 4); a('gmask', 8)
    a('real', 16); a('stabb', 16); a('vmask', 16)
    for n in ('hln_w', 'hln_b', 'hb1'):
        a(n, 4)
    a('hb2', 1)
    a('cln', 1)
    a('clog', 1)
    a('ctiny', 1)
    for l in range(DEPTH):
        for n in ('ln1w', 'ln1b', 'ln2w', 'ln2b', 'bq', 'bk', 'bv', 'bo', 'fb2'):
            a(f'{n}{l}', 4)
        a(f'fb1{l}', 16)
    return cols, idx


COLS, NCOL = build_cols()


def pack_chw(inp, real01, vmask01):
    chw = np.zeros((P, NCOL), np.float32)

    def put(name, vec):
        vec = np.asarray(vec, np.float32).reshape(-1)
        c0 = COLS[name]
        for c in range((len(vec) + P - 1) // P):
            seg = vec[c * P:(c + 1) * P]
            chw[:len(seg), c0 + c] = seg

    put('gn_w', inp['gn_w']); put('gn_b', inp['gn_b'])
    put('lnt_w', inp['lnt_w']); put('lnt_b', inp['lnt_b'])
    put('b1', inp['conv1_b']); put('b2', inp['conv2_b']); put('b3', inp['conv3_b'])
    gm = np.zeros((P, 8), np.float32)
    for p in range(P):
        gm[p, p // 16] = 1.0
    chw[:, COLS['gmask']:COLS['gmask'] + 8] = gm
    chw[:, COLS['real']:COLS['real'] + 16] = real01
    chw[:, COLS['stabb']:COLS['stabb'] + 16] = (real01 - 1.0) * 1e30
    chw[:, COLS['vmask']:COLS['vmask'] + 16] = vmask01
    put('hln_w', inp['h_ln_w']); put('hln_b', inp['h_ln_b'])
    put('hb1', inp['h_b1']); put('hb2', inp['h_b2'])
    chw[:, COLS['cln']] = LN_EPS
    chw[:, COLS['clog']] = LOG_EPS
    chw[:, COLS['ctiny']] = 1e-30
    for l in range(DEPTH):
        put(f'ln1w{l}', inp['ln1_w'][l]); put(f'ln1b{l}', inp['ln1_b'][l])
        put(f'ln2w{l}', inp['ln2_w'][l]); put(f'ln2b{l}', inp['ln2_b'][l])
        put(f'bq{l}', inp['bq'][l]); put(f'bk{l}', inp['bk'][l])
        put(f'bv{l}', inp['bv'][l]); put(f'bo{l}', inp['bo'][l])
        put(f'fb1{l}', inp['ff_b1'][l]); put(f'fb2{l}', inp['ff_b2'][l])
    return chw


def blocks(width, bs=BLK, off0=0):
    out, off = [], 0
    while off < width:
        out.append((off0 + off, min(bs, width - off)))
        off += bs
    return out


# ---------------------------------------------------------------- device build
def build(nc):
    dp = lambda n, sh, dt=F32: nc.declare_dram_parameter(n, sh, dt, isOutput=False)
    xh_d = dp('xh', (2, 36, NPAD))
    add_d = dp('addt', (2, D, NPAD))
    w1t_d = dp('w1t', (36, 64))
    w2t_d = dp('w2t', (64, 9, 128))
    w3t_d = dp('w3t', (128, D))
    chw_d = dp('chw', (P, NCOL))
    gmt_d = dp('gmt', (8, P))
    proj_d = dp('projt', (DEPTH, DH, M))
    wq_d = dp('wqb', (DEPTH, D, D), BF16)
    wk_d = dp('wkb', (DEPTH, D, D), BF16)
    wv_d = dp('wvb', (DEPTH, D, D), BF16)
    wob_d = dp('wob', (DEPTH, D, D), BF16)
    f1_d = dp('f1b', (DEPTH, D, 4 * D), BF16)
    f2_d = dp('f2b', (DEPTH, 4 * D, D), BF16)
    hw1_d = dp('hw1b', (D, D), BF16)
    hw2_d = dp('hw2', (D, 1))
    out_d = nc.declare_dram_parameter('o', (1, 2), F32, isOutput=True)

    with tile.TileContext(nc) as tc, ExitStack() as ctx:
        const = ctx.enter_context(tc.tile_pool(name='const', bufs=1))
        sp = ctx.enter_context(tc.tile_pool(name='sp', bufs=1))
        actp = ctx.enter_context(tc.tile_pool(name='actp', bufs=1))
        wpp = ctx.enter_context(tc.tile_pool(name='wpp', bufs=2))
        fp = ctx.enter_context(tc.tile_pool(name='fp', bufs=2))
        kvp = ctx.enter_context(tc.tile_pool(name='kvp', bufs=3))
        dram = ctx.enter_context(tc.tile_pool(name='dram', bufs=2, space='DRAM'))
        pmm = ctx.enter_context(tc.tile_pool(name='pmm', bufs=4, space='PSUM'))
        ps1 = ctx.enter_context(tc.tile_pool(name='ps1', bufs=3, space='PSUM'))

        # ---- constants
        chw = const.tile([P, NCOL], F32, name='chw')
        nc.sync.dma_start(chw[:], chw_d[:])
        cw = lambda name, off=0: chw[:, COLS[name] + off:COLS[name] + off + 1]
        cwp = lambda name, parts: chw[0:parts, COLS[name]:COLS[name] + 1]
        ident = const.tile([P, P], F32, name='ident')
        make_identity(nc, ident)
        identb = const.tile([P, P], BF16, name='identb')
        nc.vector.tensor_copy(identb[:], ident[:])
        ones = const.tile([P, 1], F32, name='ones')
        nc.vector.memset(ones[:], 1.0)
        onesb = const.tile([P, 1], BF16, name='onesb')
        nc.vector.memset(onesb[:], 1.0)
        onesD = const.tile([P, 1], F32, name='onesD')
        nc.vector.memset(onesD[:], 1.0 / D)
        onesDb = const.tile([P, 1], BF16, name='onesDb')
        nc.vector.memset(onesDb[:], 1.0 / D)
        gmt = const.tile([8, P], F32, name='gmt')
        nc.sync.dma_start(gmt[:], gmt_d[:])
        w1t = const.tile([36, 64], F32, name='w1t')
        nc.sync.dma_start(w1t[:], w1t_d[:])
        w2t = const.tile([64, 9, 128], F32, name='w2t')
        nc.sync.dma_start(w2t[:], w2t_d[:])
        w3t = const.tile([128, D], F32, name='w3t')
        nc.sync.dma_start(w3t[:], w3t_d[:])

        S = [sp.tile([P, N2], F32, tag=f'S{c}', name=f'S{c}') for c in range(DC)]

        # -------------------------------------------------------- shared LN
        def ln_fm(X, wc, bc, col0, width, ytag):
            """Per-token LN over the 512 partition dim (feature-major).

            X tiles are f32; returns bf16 normed tiles."""
            Y = [actp.tile([P, N2], BF16, tag=f'{ytag}{c}', name=f'{ytag}{c}')
                 for c in range(DC)]
            for c in range(DC):
                nc.vector.tensor_mul(Y[c][:, col0:col0 + width],
                                     X[c][:, col0:col0 + width],
                                     X[c][:, col0:col0 + width])
            srow = fp.tile([1, N2], F32, tag='srow', bufs=1, name='srow')
            qrow = fp.tile([1, N2], F32, tag='qrow', bufs=1, name='qrow')
            mrow, vrow = srow, qrow
            for o, bw in blocks(width, BLK, col0):
                ps = ps1.tile([1, BLK], F32, tag='ps1', name='ps')
                pq = ps1.tile([1, BLK], F32, tag='ps1', name='pq')
                for c in range(DC):
                    nc.tensor.matmul(ps[:, :bw], onesD[:], X[c][:, o:o + bw],
                                     start=(c == 0), stop=(c == DC - 1))
                    nc.tensor.matmul(pq[:, :bw], onesDb[:], Y[c][:, o:o + bw],
                                     start=(c == 0), stop=(c == DC - 1))
                nc.any.tensor_copy(srow[:, o:o + bw], ps[:, :bw])
                nc.any.tensor_copy(qrow[:, o:o + bw], pq[:, :bw])
            sl = slice(col0, col0 + width)
            MU = actp.tile([P, N2], F32, tag='MU', name='MU')
            RS = actp.tile([P, N2], F32, tag='RS', name='RS')
            trow = MU[0:1, :]
            nc.vector.tensor_mul(trow[:, sl], srow[:, sl], srow[:, sl])
            nc.vector.tensor_sub(vrow[:, sl], qrow[:, sl], trow[:, sl])
            nc.scalar.activation(vrow[:, sl], vrow[:, sl], AF.Ln,
                                 bias=cwp('cln', 1))
            nc.scalar.activation(vrow[:, sl], vrow[:, sl], AF.Exp, scale=-0.5)
            nc.gpsimd.partition_broadcast(MU[:, sl], mrow[:, sl], P)
            nc.gpsimd.partition_broadcast(RS[:, sl], vrow[:, sl], P)
            for c in range(DC):
                nc.vector.tensor_sub(Y[c][:, sl], X[c][:, sl], MU[:, sl])
                nc.vector.tensor_mul(Y[c][:, sl], Y[c][:, sl], RS[:, sl])
                nc.scalar.activation(Y[c][:, sl], Y[c][:, sl], AF.Identity,
                                     scale=cw(wc, c), bias=cw(bc, c))
            return Y

        # full-width projection: dst[128, N2] = (w^T y) + bias, both heads of
        # a pair. kc-outer so each stationary is loaded once per 4 blocks.
        def proj_mm(wd, l, hp, bn, dst):
            wt = wpp.tile([P, DC, P], BF16, tag='wpq', name='wt')
            nc.sync.dma_start(
                wt[:], wd[l, :, hp * P:(hp + 1) * P]
                .rearrange('(kc p) m -> p kc m', p=P))
            bias = chw[:, COLS[f'{bn}{l}'] + hp:COLS[f'{bn}{l}'] + hp + 1]
            pms = [pmm.tile([P, BLK], F32, tag='pmm', name=f'pm{bi}')
                   for bi in range(4)]
            for kc in range(DC):
                for bi, (b, bw) in enumerate(blocks(N2)):
                    nc.tensor.matmul(pms[bi][:], wt[:, kc, :], y[kc][:, b:b + bw],
                                     start=(kc == 0), stop=(kc == DC - 1))
            for bi, (b, bw) in enumerate(blocks(N2)):
                nc.scalar.activation(dst[:, b:b + bw], pms[bi][:], AF.Identity,
                                     bias=bias)

        # -------------------------------------------------------- tokenizer
        for s in range(2):
            for c in range(DC):
                nc.sync.dma_start(S[c][:, s * NPAD:(s + 1) * NPAD],
                                  add_d[s, c * P:(c + 1) * P, :])
        for s in range(2):
            t1 = fp.tile([36, NPAD], F32, tag='tokA', bufs=1, name='t1')
            nc.sync.dma_start(t1[:], xh_d[s])
            y1h = fp.tile([64, L + 8], F32, tag='tokB', bufs=1, name='y1h')
            nc.vector.memset(y1h[:], 0.0)
            for o, bw in blocks(L):
                p1 = pmm.tile([64, BLK], F32, tag='pmm', name='p1')
                nc.tensor.matmul(p1[:, :bw], w1t[:], t1[:, o:o + bw],
                                 start=True, stop=True)
                nc.scalar.activation(y1h[:, 4 + o:4 + o + bw], p1[:, :bw], AF.Gelu,
                                     bias=chw[0:64, COLS['b1']:COLS['b1'] + 1])
            y2 = fp.tile([P, NPAD], F32, tag='tokA', bufs=1, name='y2')
            for o, bw in blocks(L):
                p2 = pmm.tile([P, BLK], F32, tag='pmm', name='p2')
                for t in range(9):
                    nc.tensor.matmul(p2[:, :bw], w2t[:, t, :],
                                     y1h[:, t + o:t + o + bw],
                                     start=(t == 0), stop=(t == 8))
                nc.scalar.activation(y2[:, o:o + bw], p2[:, :bw], AF.Gelu,
                                     bias=cw('b2'))
            x3 = [actp.tile([P, NPAD], F32, tag=f'A{c}', name=f'x3{c}')
                  for c in range(DC)]
            for c in range(DC):
                for o, bw in blocks(L):
                    p3 = pmm.tile([P, BLK], F32, tag='pmm', name='p3')
                    nc.tensor.matmul(p3[:, :bw], w3t[:, c * P:(c + 1) * P],
                                     y2[:, o:o + bw], start=True, stop=True)
                    nc.scalar.activation(x3[c][:, o:o + bw], p3[:, :bw],
                                         AF.Identity, bias=cw('b3', c))
            # GroupNorm(32, 512) over [16ch x 1000]
            stats = fp.tile([P, 8], F32, tag='gstats', name='stats')
            sqt = fp.tile([P, NPAD], F32, tag='tokB', bufs=1, name='sqt')
            for c in range(DC):
                nc.vector.tensor_reduce(stats[:, c:c + 1], x3[c][:, 0:L], AX.X, OP.add)
                nc.vector.tensor_mul(sqt[:, 0:L], x3[c][:, 0:L], x3[c][:, 0:L])
                nc.vector.tensor_reduce(stats[:, 4 + c:5 + c], sqt[:, 0:L], AX.X, OP.add)
            pg = ps1.tile([8, 8], F32, tag='ps1', name='pg')
            nc.tensor.matmul(pg[:], chw[:, COLS['gmask']:COLS['gmask'] + 8],
                             stats[:], start=True, stop=True)
            gs = fp.tile([8, 8], F32, tag='gs', name='gs')
            nc.vector.tensor_scalar_mul(gs[:], pg[:], 1.0 / (16 * L))
            gm2 = fp.tile([8, 4], F32, tag='gm2', name='gm2')
            nc.vector.tensor_mul(gm2[:], gs[:, 0:4], gs[:, 0:4])
            nc.vector.tensor_sub(gs[:, 4:8], gs[:, 4:8], gm2[:])
            nc.scalar.activation(gs[:, 4:8], gs[:, 4:8], AF.Ln, bias=cwp('cln', 8))
            nc.scalar.activation(gs[:, 4:8], gs[:, 4:8], AF.Exp, scale=-0.5)
            pb = ps1.tile([P, 8], F32, tag='ps1', name='pb')
            nc.tensor.matmul(pb[:], gmt[:], gs[:], start=True, stop=True)
            cstat = fp.tile([P, 8], F32, tag='cstat', name='cstat')
            nc.vector.tensor_copy(cstat[:], pb[:])
            for c in range(DC):
                nc.vector.tensor_scalar(x3[c][:, 0:L], x3[c][:, 0:L],
                                        cstat[:, c:c + 1], cstat[:, 4 + c:5 + c],
                                        OP.subtract, OP.mult)
                nc.scalar.activation(x3[c][:, 0:L], x3[c][:, 0:L], AF.Identity,
                                     scale=cw('gn_w', c), bias=cw('gn_b', c))
            tok = ln_fm(x3, 'lnt_w', 'lnt_b', 0, L, 'y')
            b0 = s * NPAD
            for c in range(DC):
                nc.vector.tensor_add(S[c][:, b0 + 1:b0 + 1 + L],
                                     S[c][:, b0 + 1:b0 + 1 + L], tok[c][:, 0:L])

        # -------------------------------------------------------- layers
        for l in range(DEPTH):
            projT = fp.tile([DH, M], F32, tag='projT', name='projT')
            nc.sync.dma_start(projT[:], proj_d[l])
            # duplicated into both partition halves so head-1 slices
            # (base partition 64) can pair with it in matmuls; the second
            # half is filled by DMA (engines can't shift partitions)
            projTb = fp.tile([P, M], BF16, tag='projTb', name='projTb')
            nc.vector.tensor_copy(projTb[0:DH, :], projT[:])
            nc.sync.dma_start(projTb[DH:P, :], projTb[0:DH, :])

            y = ln_fm(S, f'ln1w{l}', f'ln1b{l}', 0, N2, 'y')

            # per-layer k-side context accumulators [65, 272] x 16 head-samples
            ctxE = fp.tile([65, 16, 272], BF16, tag='ctxE', bufs=1, name='ctxE')
            smax = fp.tile([P, 16], F32, tag='smax', name='smax')

            # ---- phase K: k/v projections, kp, ctx/ksum, local stab byproduct
            for hp in range(4):
                kh2 = fp.tile([P, N2], BF16, tag='kh2', bufs=2, name='kh2')
                vh2 = fp.tile([P, N2], BF16, tag='vh2', bufs=2, name='vh2')
                proj_mm(wk_d, l, hp, 'bk', kh2)
                proj_mm(wv_d, l, hp, 'bv', vh2)
                for hh in range(2):
                    hsl = slice(hh * DH, (hh + 1) * DH)
                    for s in range(2):
                        base = s * NPAD
                        idx = hp * 4 + hh * 2 + s
                        # --- -diag_k per token, [P, 8]; pad rows get -1e30
                        biask = fp.tile([P, 8], F32, tag='biask', name='biask')
                        pkt = ps1.tile([P, 512], BF16, tag='ps1', name='pkt')
                        for t in range(NT):
                            csl = slice(base + t * P, base + (t + 1) * P)
                            nc.tensor.transpose(pkt[:, t * 64:(t + 1) * 64],
                                                kh2[hsl, csl], identb[hsl, hsl])
                        sqk = fp.tile([P, 512], F32, tag='sqk', name='sqk')
                        nc.scalar.activation(sqk[:], pkt[:], AF.Square,
                                             scale=DNS)
                        for t in range(NT):
                            nc.vector.tensor_reduce(biask[:, t:t + 1],
                                                    sqk[:, t * 64:(t + 1) * 64],
                                                    AX.X, OP.add)
                        nc.vector.tensor_scalar_mul(biask[:], biask[:], -1.0)
                        bmask = fp.tile([P, 8], F32, tag='bmask', name='bmask')
                        nc.vector.tensor_add(
                            bmask[:], biask[:],
                            chw[:, COLS['stabb'] + s * 8:COLS['stabb'] + s * 8 + 8])
                        # --- kp tiles + ctx'^T [65, 267] accumulation
                        pctx = ps1.tile([65, 272], F32, tag='psx', bufs=1,
                                        name='pctx')
                        rm8 = fp.tile([P, 8], F32, tag='rm8', name='rm8')
                        for t in range(NT):
                            csl = slice(base + t * P, base + (t + 1) * P)
                            pdk = ps1.tile([P, 272], F32, tag='ps1', name='pdk')
                            nc.tensor.matmul(pdk[:, 0:M], kh2[hsl, csl],
                                             projTb[hsl, :], start=True, stop=True)
                            kp = kvp.tile([P, 272], BF16, tag='kp', name='kp')
                            nc.scalar.activation(kp[:, 0:M], pdk[:, 0:M], AF.Exp,
                                                 bias=bmask[:, t:t + 1])
                            nc.vector.tensor_copy(kp[:, M:M + 1],
                                                  cw('real', s * 8 + t))
                            nc.vector.tensor_reduce(rm8[:, t:t + 1], kp[:, 0:M],
                                                    AX.X, OP.max)
                            pvt = ps1.tile([P, 64], BF16, tag='ps1', name='pvt')
                            nc.tensor.transpose(pvt[:], vh2[hsl, csl],
                                                identb[hsl, hsl])
                            v1 = kvp.tile([P, 65], BF16, tag='v1', name='v1')
                            nc.vector.tensor_scalar_mul(v1[:, 0:64], pvt[:],
                                                        cw('vmask', s * 8 + t))
                            nc.vector.memset(v1[:, 64:65], 1.0)
                            nc.tensor.matmul(pctx[:, 0:M + 1], v1[:], kp[:, 0:M + 1],
                                             start=(t == 0), stop=(t == NT - 1))
                        nc.vector.tensor_copy(ctxE[:, idx, 0:M + 1],
                                              pctx[:, 0:M + 1])
                        # --- local stab byproduct, kept in the exp domain:
                        # max_m e^{dd} = (max_m kp) * e^{diag}. Exp shares the
                        # kp table (no ACT table thrash); pad rows stay 0.
                        m8 = fp.tile([P, 8], F32, tag='m8', name='m8')
                        nc.scalar.activation(m8[:], biask[:], AF.Exp,
                                             scale=-1.0)
                        nc.vector.tensor_mul(m8[:], m8[:], rm8[:])
                        nc.vector.tensor_reduce(smax[:, idx:idx + 1], m8[:],
                                                AX.X, OP.max)

            # ---- global key-stab: AllReduce(max) across the 8 cores.
            # Overlaps with the q-side below (consumed only at ctxT/vsc).
            sfin = fp.tile([P, 1], F32, tag='sfin', name='sfin')
            nc.vector.tensor_reduce(sfin[:], smax[:], AX.X, OP.max)
            nc.gpsimd.partition_all_reduce(sfin[:], sfin[:], P, ReduceOp.max)
            bin_ = dram.tile([P, 1], F32, name='bin')
            bout = dram.tile([P, 1], F32, name='bout')
            nc.sync.dma_start(bin_[:], sfin[:])
            nc.gpsimd.collective_compute(
                'AllReduce', OP.max,
                replica_groups=[list(range(N_CORES))],
                ins=[bin_.opt()], outs=[bout.opt()])
            stabg = fp.tile([P, 1], F32, tag='stabg', name='stabg')
            nc.sync.dma_start(stabg[:], bout[:])
            # stabg already holds e^{stab_g}; gamma = eps * e^{stab_g}
            ceps65 = fp.tile([65, 1], F32, tag='ceps65', name='ceps65')
            nc.vector.tensor_scalar_mul(ceps65[:], stabg[0:65, :], 1e-4)

            # ---- phase Q: q projection, qp, eps corrections, num/den -> A
            A = [actp.tile([P, N2], BF16, tag=f'A{c}', name=f'Aa{c}')
                 for c in range(DC)]
            for hp in range(4):
                qh2 = fp.tile([P, N2], BF16, tag='qh2', bufs=2, name='qh2')
                proj_mm(wq_d, l, hp, 'bq', qh2)
                # (dn^2/2)*q^2 for diag_q, both heads/samples in one op
                sqq = fp.tile([P, N2], BF16, tag='sqq', bufs=1, name='sqq')
                nc.scalar.activation(sqq[:], qh2[:], AF.Square, scale=DNS)
                for hh in range(2):
                    hsl = slice(hh * DH, (hh + 1) * DH)
                    for s in range(2):
                        base = s * NPAD
                        idx = hp * 4 + hh * 2 + s
                        # --- qp = exp(ddq), feature-major
                        qp = [fp.tile([P, NPAD], BF16, tag='qp0', bufs=2, name='qp0'),
                              fp.tile([P, NPAD], BF16, tag='qp1', bufs=2, name='qp1'),
                              fp.tile([10, NPAD], BF16, tag='qp2', bufs=2, name='qp2')]
                        for ci, (m0, mw) in enumerate(MCH):
                            for b, bw in blocks(NPAD):
                                pdq = pmm.tile([P, BLK], F32, tag='pmm', name='pdq')
                                nc.tensor.matmul(
                                    pdq[0:mw, :], projTb[hsl, m0:m0 + mw],
                                    qh2[hsl, base + b:base + b + bw],
                                    start=True, stop=True)
                                nc.scalar.activation(qp[ci][0:mw, b:b + bw],
                                                     pdq[0:mw, :], AF.Exp)
                        # --- colmax = e^{stab_q} per token: fold the 3 chunks
                        # with same-partition DVE maxes, then ONE gpsimd
                        # all-reduce (was three)
                        ar = fp.tile([P, NPAD], BF16, tag='ar', bufs=2, name='ar')
                        nc.vector.tensor_tensor(ar[:], qp[0][:], qp[1][:], OP.max)
                        nc.vector.tensor_tensor(ar[0:10, :], ar[0:10, :],
                                                qp[2][:], OP.max)
                        nc.gpsimd.partition_all_reduce(ar[:], ar[:], P,
                                                       ReduceOp.max)
                        # --- epsE = eps * e^{diag_q} * colmax
                        epsE = fp.tile([1, NPAD], BF16, tag='epsE', bufs=2,
                                       name='epsE')
                        for b, bw in blocks(NPAD):
                            pq2 = ps1.tile([1, BLK], F32, tag='ps1', name='pq2')
                            nc.tensor.matmul(pq2[:, :bw], onesb[hsl, :],
                                             sqq[hsl, base + b:base + b + bw],
                                             start=True, stop=True)
                            nc.scalar.activation(epsE[:, b:b + bw], pq2[:, :bw],
                                                 AF.Exp, bias=cwp('clog', 1))
                        nc.vector.tensor_mul(epsE[:], epsE[:], ar[0:1, :])
                        # --- ctxT = ctxE + gamma * vsum; -> [266(+1), 65] chunks
                        vsc = fp.tile([65, 1], F32, tag='vsc', name='vsc')
                        nc.vector.tensor_mul(vsc[:], ctxE[:, idx, M:M + 1],
                                             ceps65[:])
                        ctxT = fp.tile([65, M], BF16, tag='ctxT', name='ctxT')
                        nc.vector.tensor_scalar(ctxT[:], ctxE[:, idx, 0:M],
                                                vsc[:], None, OP.add)
                        ctx_sb = []
                        for ci, (m0, mw) in enumerate(MCH):
                            ptc = ps1.tile([P, 65], BF16, tag='ps1', name='ptc')
                            nc.tensor.transpose(ptc[0:mw, :], ctxT[:, m0:m0 + mw],
                                                identb[0:65, 0:65])
                            csb = fp.tile([P, 65], BF16, tag=f'ctx{ci}', name=f'c{ci}')
                            nc.any.tensor_copy(csb[0:mw, :], ptc[0:mw, :])
                            ctx_sb.append(csb)
                        # csr = colsum of ctxT, for the rank-1 eps term
                        pcs = ps1.tile([1, 65], F32, tag='ps1', name='pcs')
                        for ci, (m0, mw) in enumerate(MCH):
                            nc.tensor.matmul(pcs[:], onesb[0:mw, :],
                                             ctx_sb[ci][0:mw, :],
                                             start=(ci == 0), stop=(ci == 2))
                        csr = fp.tile([1, 65], BF16, tag='csr', name='csr')
                        nc.any.tensor_copy(csr[:], pcs[:])
                        # --- num_den [65, n]; rows 0..63 num, row 64 den
                        for b, bw in blocks(NPAD):
                            pnd = ps1.tile([65, BLK], F32, tag='ps1', name='pnd')
                            for ci, (m0, mw) in enumerate(MCH):
                                nc.tensor.matmul(pnd[:], ctx_sb[ci][0:mw, :],
                                                 qp[ci][0:mw, b:b + bw],
                                                 start=(ci == 0), stop=False)
                            nc.tensor.matmul(pnd[:], csr[:], epsE[:, b:b + bw],
                                             start=False, stop=True)
                            den = fp.tile([1, BLK], F32, tag='den', bufs=2,
                                          name='den')
                            nc.vector.tensor_copy(den[:], pnd[64:65, :])
                            dinv = fp.tile([1, BLK], F32, tag='dinv', bufs=2,
                                           name='dinv')
                            nc.vector.reciprocal_approx_fast(dinv[:], den[:])
                            dvb = fp.tile([64, BLK], F32, tag='dvb', bufs=2,
                                          name='dvb')
                            nc.gpsimd.partition_broadcast(dvb[:], dinv[:], 64)
                            nc.vector.tensor_mul(
                                A[hp][hsl, base + b:base + b + bw],
                                pnd[0:64, :], dvb[:])

            # ---- wo: S += A @ wo + bo (kc-outer for stationary reuse)
            for mc in range(DC):
                wt = wpp.tile([P, DC, P], BF16, tag='wpo', name='wto')
                nc.sync.dma_start(
                    wt[:], wob_d[l, :, mc * P:(mc + 1) * P]
                    .rearrange('(kc p) m -> p kc m', p=P))
                pms = [pmm.tile([P, BLK], F32, tag='pmm', name=f'pmo{bi}')
                       for bi in range(4)]
                for kc in range(DC):
                    for bi, (b, bw) in enumerate(blocks(N2)):
                        nc.tensor.matmul(pms[bi][:], wt[:, kc, :],
                                         A[kc][:, b:b + bw],
                                         start=(kc == 0), stop=(kc == DC - 1))
                for bi, (b, bw) in enumerate(blocks(N2)):
                    nc.vector.tensor_add(S[mc][:, b:b + bw], S[mc][:, b:b + bw],
                                         pms[bi][:])
                nc.scalar.activation(S[mc][:], S[mc][:], AF.Identity,
                                     bias=cw(f'bo{l}', mc))

            # ---- FF in quarters of the 2048 hidden dim (kc-outer)
            y2t = ln_fm(S, f'ln2w{l}', f'ln2b{l}', 0, N2, 'y')
            for q in range(4):
                w1q = fp.tile([P, DC, BLK], BF16, tag='w1q', bufs=1, name='w1q')
                nc.sync.dma_start(
                    w1q[:], f1_d[l, :, q * BLK:(q + 1) * BLK]
                    .rearrange('(kc p) m -> p kc m', p=P))
                w2q = fp.tile([P, DC, BLK], BF16, tag='w2q', bufs=1, name='w2q')
                nc.sync.dma_start(
                    w2q[:], f2_d[l, q * BLK:(q + 1) * BLK, :]
                    .rearrange('(kc p) m -> p kc m', p=P))
                # reuse the dead A tiles (same shape) for the FF hidden
                H = [actp.tile([P, N2], BF16, tag=f'A{mc}', name=f'H{mc}')
                     for mc in range(DC)]
                for mc in range(DC):
                    pms = [pmm.tile([P, BLK], F32, tag='pmm', name=f'pmf1{bi}')
                           for bi in range(4)]
                    for kc in range(DC):
                        for bi, (b, bw) in enumerate(blocks(N2)):
                            nc.tensor.matmul(pms[bi][:],
                                             w1q[:, kc, mc * P:(mc + 1) * P],
                                             y2t[kc][:, b:b + bw],
                                             start=(kc == 0), stop=(kc == DC - 1))
                    for bi, (b, bw) in enumerate(blocks(N2)):
                        nc.scalar.activation(H[mc][:, b:b + bw], pms[bi][:],
                                             AF.Gelu, bias=cw(f'fb1{l}', q * 4 + mc))
                for mc in range(DC):
                    pms = [pmm.tile([P, BLK], F32, tag='pmm', name=f'pmf2{bi}')
                           for bi in range(4)]
                    for kc in range(DC):
                        for bi, (b, bw) in enumerate(blocks(N2)):
                            nc.tensor.matmul(pms[bi][:],
                                             w2q[:, kc, mc * P:(mc + 1) * P],
                                             H[kc][:, b:b + bw],
                                             start=(kc == 0), stop=(kc == DC - 1))
                    for bi, (b, bw) in enumerate(blocks(N2)):
                        nc.vector.tensor_add(S[mc][:, b:b + bw],
                                             S[mc][:, b:b + bw], pms[bi][:])
            for mc in range(DC):
                nc.scalar.activation(S[mc][:], S[mc][:], AF.Identity,
                                     bias=cw(f'fb2{l}', mc))
                for s in range(2):
                    nc.vector.memset(S[mc][:, s * NPAD + 1 + L:(s + 1) * NPAD], 0.0)

        # -------------------------------------------------------- head
        clsx = [fp.tile([P, 2], F32, tag=f'cls{c}', name=f'cls{c}')
                for c in range(DC)]
        for c in range(DC):
            nc.vector.tensor_copy(clsx[c][:, 0:1], S[c][:, 0:1])
            nc.vector.tensor_copy(clsx[c][:, 1:2], S[c][:, NPAD:NPAD + 1])
        hx = ln_fm(clsx, 'hln_w', 'hln_b', 0, 2, 'y')
        hh_t = []
        for mc in range(DC):
            wt = wpp.tile([P, DC, P], BF16, tag='wpq', name='wth')
            nc.sync.dma_start(wt[:], hw1_d[:, mc * P:(mc + 1) * P]
                              .rearrange('(kc p) m -> p kc m', p=P))
            pm = ps1.tile([P, 2], F32, tag='ps1', name='pmh')
            for kc in range(DC):
                nc.tensor.matmul(pm[:], wt[:, kc, :], hx[kc][:, 0:2],
                                 start=(kc == 0), stop=(kc == DC - 1))
            ht = fp.tile([P, 2], F32, tag=f'hh{mc}', name=f'hhd{mc}')
            nc.scalar.activation(ht[:], pm[:], AF.Gelu, bias=cw('hb1', mc))
            hh_t.append(ht)
        wt2 = fp.tile([P, DC, 1], F32, tag='wt2', name='wt2')
        nc.sync.dma_start(wt2[:], hw2_d[:, :].rearrange('(kc p) m -> p kc m', p=P))
        po = ps1.tile([1, 2], F32, tag='ps1', name='po')
        for kc in range(DC):
            nc.tensor.matmul(po[:], wt2[:, kc, :], hh_t[kc][:, 0:2],
                             start=(kc == 0), stop=(kc == DC - 1))
        osb = fp.tile([1, 2], F32, tag='osb', name='osb')
        nc.scalar.activation(osb[:], po[:], AF.Identity,
                             bias=chw[0:1, COLS['hb2']:COLS['hb2'] + 1])
        nc.sync.dma_start(out_d[:], osb[:])

    return nc


# ---------------------------------------------------------------- host wrapper
def kernel(**inputs):
    inp = {k: np.asarray(v) for k, v in inputs.items()}
    B = inp['sig_n'].shape[0]
    assert B == 16, f'expected B=16, got {B}'

    sig = inp['sig_n'].astype(np.float32)
    x = np.where(np.isfinite(sig), sig, 0.0)
    x = np.where(x == -1.0, 0.0, x).astype(np.float32)
    valid = np.any(sig != -1.0, axis=1)                # [16, 1000]

    xh = np.zeros((B, 4, L + 8), np.float32)
    xh[:, :, 4:4 + L] = x
    t1full = np.zeros((B, 36, NPAD), np.float32)
    for t in range(9):
        t1full[:, 4 * t:4 * t + 4, 0:L] = xh[:, :, t:t + L]

    meta = inp['meta'].astype(np.int64)
    e_chr = inp['emb_chr'][np.clip(meta[:, 2], 0, 22)]
    e_gene = inp['emb_gene'][np.maximum(inp['gene_id'].astype(np.int64), 0)]
    e_exon = inp['emb_exon'][np.clip(inp['exon_id'].astype(np.int64), 0, 128)]
    e_ctx = (e_chr + e_gene + e_exon).astype(np.float32)

    pos = np.arange(L, dtype=np.float32)[:, None]
    div = np.exp(np.arange(0, D, 2, dtype=np.float32) * (-np.log(10000.0) / D))
    pe = np.zeros((L, D), np.float32)
    pe[:, 0::2] = np.sin(pos * div)
    pe[:, 1::2] = np.cos(pos * div)

    ADD = np.zeros((B, D, NPAD), np.float32)
    ADD[:, :, 0] = inp['cls'][0, 0][None, :] + e_ctx
    ADD[:, :, 1:1 + L] = pe.T[None] + e_ctx[:, :, None]

    w1t = np.zeros((36, 64), np.float32)
    for t in range(9):
        w1t[4 * t:4 * t + 4] = inp['conv1_w'][:, :, t].T
    w2t = np.ascontiguousarray(inp['conv2_w'].transpose(1, 2, 0)).astype(np.float32)
    w3t = np.ascontiguousarray(inp['conv3_w'][:, :, 0].T).astype(np.float32)
    gmt = np.zeros((8, P), np.float32)
    for p in range(P):
        gmt[p // 16, p] = 1.0
    projt = np.ascontiguousarray((inp['proj'] * DN).transpose(0, 2, 1)).astype(np.float32)

    bf = lambda a: np.ascontiguousarray(np.asarray(a, np.float32).astype(ml_dtypes.bfloat16))
    shared = dict(
        w1t=w1t, w2t=w2t, w3t=w3t, gmt=gmt, projt=projt,
        wqb=bf(inp['wq']), wkb=bf(inp['wk']), wvb=bf(inp['wv']),
        wob=bf(inp['wo']),
        f1b=bf(inp['ff_w1']), f2b=bf(inp['ff_w2']),
        hw1b=bf(inp['h_w1']),
        hw2=np.ascontiguousarray(inp['h_w2'], dtype=np.float32),
    )

    in_maps = []
    for c in range(N_CORES):
        b0 = 2 * c
        real01 = np.zeros((P, 16), np.float32)
        vm01 = np.zeros((P, 16), np.float32)
        for s in range(2):
            for n in range(NPAD):
                t, row = n // P, n % P
                if n <= L:
                    real01[row, s * 8 + t] = 1.0
                    if n == 0 or valid[b0 + s, n - 1]:
                        vm01[row, s * 8 + t] = 1.0
        chw = pack_chw(inp, real01, vm01)
        in_maps.append(dict(
            shared,
            xh=np.ascontiguousarray(t1full[b0:b0 + 2]),
            addt=np.ascontiguousarray(ADD[b0:b0 + 2]),
            chw=chw,
        ))

    nc = bacc.Bacc()
    build(nc)
    nc.finalize()
    res = run_bass_kernel_spmd(nc, in_maps, list(range(N_CORES)))
    global LAST_RESULT
    LAST_RESULT = res
    out = np.concatenate([np.asarray(res.results[c]['o']).reshape(2)
                          for c in range(N_CORES)])
    return out.astype(np.float32)


LAST_RESULT = None


if __name__ == '__main__':
    import reference
    inputs = {k: np.asarray(v) for k, v in reference.setup_inputs().items()}
    got = kernel(**inputs)
    print('kernel out:', got)
